# revision 1
# baseline (speedup 1.0000x reference)
"""Trainium2 Bass kernel for a binarized-conv ResNet BasicBlock.

    y1 = conv3x3(x, sign(w1)); out1 = relu(BN(y1))
    y2 = conv3x3(out1, sign(w2)); out = relu(BN(y2) + x)

BN is training-mode (batch stats over N,H,W). Sharding: data-parallel over
the batch (N=32 -> 4 images per core on 8 cores); conv weights + BN params
replicated; BN stats sync'd with a tiny [128,2] AllReduce (sum, sumsq).

Conv mapping: C_in=128 lands exactly on the 128 SBUF partitions; a 3x3
conv is 9 shifted matmuls accumulated in PSUM (lhsT = [Cin, Cout] per tap,
rhs = padded input rows). Matmul inputs are bf16 (weights are exactly
+/-1); accumulation is fp32 in PSUM.
"""

import numpy as np

import concourse.bass as bass
import concourse.tile as tile
from concourse import bacc, mybir
from concourse.bass_utils import run_bass_kernel_spmd

F32 = mybir.dt.float32
BF16 = mybir.dt.bfloat16
NP_BF16 = mybir.dt.np(BF16)

N, C, H, W = 32, 128, 56, 56
NCORES = 8
NLOC = N // NCORES           # images per core
HP, WP = H + 2, W + 2        # padded spatial dims (halo of zeros)
R = 8                        # output rows per matmul group
NG = H // R                  # groups per image
F = R * W                    # moving free dim per matmul (448 <= 512)
CNT_GLB = N * H * W          # global BN count
EPS = 1e-5

_CACHE = {}


def _conv_phase(tc, w_sb, src_pads, dst_ys, bnst):
    """One conv layer: 9-tap matmul accumulation per (image, row-group),
    PSUM evicted to SBUF bf16 via ACT copy, then bn_stats on the evicted
    tile for the sync-BN statistics."""
    nc = tc.nc
    GB = 2  # groups per weight-stationary batch (taps outer: the PE reloads
    #         weights every GB matmuls instead of every matmul)
    groups = [(n, g) for n in range(NLOC) for g in range(NG)]
    xvs = [src_pads[n][:].rearrange("p (h w) -> p h w", w=WP) for n in range(NLOC)]
    with tc.tile_pool(name="psum", bufs=6, space="PSUM") as psum:
        for b0 in range(0, len(groups), GB):
            batch = groups[b0:b0 + GB]
            tiles = [psum.tile([C, F], F32, tag="ps", name=f"ps{b0 + i}")
                     for i in range(len(batch))]
            t = 0
            for ky in range(3):
                for kx in range(3):
                    for i, (n, g) in enumerate(batch):
                        r0 = g * R
                        nc.tensor.matmul(
                            tiles[i][:],
                            w_sb[:, t * C:(t + 1) * C],
                            xvs[n][:, r0 + ky:r0 + ky + R, kx:kx + W],
                            start=(t == 0),
                            stop=(t == 8),
                        )
                    t += 1
            for i, (n, g) in enumerate(batch):
                r0 = g * R
                seg = dst_ys[n][:][:, r0 * W:(r0 + R) * W]
                nc.scalar.copy(seg, tiles[i][:])
                nc.vector.bn_stats(bnst[:, n * NG + g, :], seg)


def _bn_coeffs(tc, pools, bnst, gamma_sb, beta_sb, eps_sb, idx, fake_cc=False):
    """Local (mean,var) -> (sum,sumsq) partials, AllReduce across the 8
    cores, then a = gamma*rsqrt(var+eps), b = beta - mean*a.  All [128,1].

    fake_cc replaces the collective with a DRAM->DRAM copy so the program
    can run under the single-core TimelineSim (timing studies only)."""
    nc = tc.nc
    small, dram = pools
    alu = mybir.AluOpType

    mv = small.tile([C, 2], F32, tag=f"mv{idx}")
    nc.vector.bn_aggr(mv[:], bnst[:])
    # partial sums: sum = mean*cnt ; sumsq = (var + mean^2)*cnt
    cnt_loc = float(NLOC * H * W)
    e2 = small.tile([C, 1], F32, tag=f"e2{idx}")
    nc.vector.scalar_tensor_tensor(
        e2[:], mv[:, 0:1], mv[:, 0:1], mv[:, 1:2], op0=alu.mult, op1=alu.add)
    part = small.tile([C, 2], F32, tag=f"part{idx}")
    nc.vector.tensor_scalar_mul(part[:, 0:1], mv[:, 0:1], cnt_loc)
    nc.vector.tensor_scalar_mul(part[:, 1:2], e2[:], cnt_loc)

    cc_in = dram.tile([C, 2], F32, tag=f"ccin{idx}")
    cc_out = dram.tile([C, 2], F32, tag=f"ccout{idx}")
    nc.sync.dma_start(cc_in[:], part[:])
    if fake_cc:
        nc.sync.dma_start(cc_out[:], cc_in[:])
    else:
        nc.gpsimd.collective_compute(
            "AllReduce",
            alu.add,
            replica_groups=[list(range(NCORES))],
            ins=[cc_in[:].opt()],
            outs=[cc_out[:].opt()],
        )
    gl = small.tile([C, 2], F32, tag=f"gl{idx}")
    nc.sync.dma_start(gl[:], cc_out[:])

    inv_cnt = 1.0 / float(CNT_GLB)
    mg = small.tile([C, 1], F32, tag=f"mg{idx}")
    e2g = small.tile([C, 1], F32, tag=f"e2g{idx}")
    nc.vector.tensor_scalar_mul(mg[:], gl[:, 0:1], inv_cnt)
    nc.vector.tensor_scalar_mul(e2g[:], gl[:, 1:2], inv_cnt)
    # negvar = mg^2 - E[y^2]; std = sqrt(-negvar + eps)
    negvar = small.tile([C, 1], F32, tag=f"negvar{idx}")
    nc.vector.scalar_tensor_tensor(
        negvar[:], mg[:], mg[:], e2g[:], op0=alu.mult, op1=alu.subtract)
    std = small.tile([C, 1], F32, tag=f"std{idx}")
    nc.scalar.activation(std[:], negvar[:], mybir.ActivationFunctionType.Sqrt,
                         bias=eps_sb[:], scale=-1.0)
    inv = small.tile([C, 1], F32, tag=f"inv{idx}")
    nc.vector.reciprocal(inv[:], std[:])
    a_t = small.tile([C, 1], F32, tag=f"a{idx}")
    nc.vector.tensor_mul(a_t[:], gamma_sb[:], inv[:])
    ma = small.tile([C, 1], F32, tag=f"ma{idx}")
    nc.vector.tensor_mul(ma[:], mg[:], a_t[:])
    b_t = small.tile([C, 1], F32, tag=f"b{idx}")
    nc.vector.tensor_tensor(b_t[:], beta_sb[:], ma[:], op=alu.subtract)
    return a_t, b_t


def _build_body(tc, xpad_d, w1_d, w2_d, g1_d, b1_d, g2_d, b2_d, out_d,
                reps=1, fake_cc=False):
    nc = tc.nc

    with (
        tc.tile_pool(name="persist", bufs=1) as persist,
        tc.tile_pool(name="small", bufs=1) as small,
        tc.tile_pool(name="dram", bufs=1, space="DRAM") as dram,
        tc.tile_pool(name="fin", bufs=6) as fin,
        tc.tile_pool(name="ostage", bufs=6) as ostage,
    ):
        pools = (persist, small, dram, fin, ostage)
        args = (xpad_d, w1_d, w2_d, g1_d, b1_d, g2_d, b2_d, out_d)
        if reps == 1:
            _emit_iteration(tc, pools, args, fake_cc)
        else:
            with tc.For_i(0, reps, 1):
                _emit_iteration(tc, pools, args, fake_cc)


def _emit_iteration(tc, pools, args, fake_cc):
    nc = tc.nc
    act = mybir.ActivationFunctionType
    alu = mybir.AluOpType
    persist, small, dram, fin, ostage = pools
    xpad_d, w1_d, w2_d, g1_d, b1_d, g2_d, b2_d, out_d = args
    if True:
        # ---- per-image persistent buffers (x load issued first: the first
        # conv group waits on image 0) ----
        xpad_sb = [persist.tile([C, HP * WP], BF16, tag=f"xp{n}", name=f"xp{n}") for n in range(NLOC)]
        o1p_sb = [persist.tile([C, HP * WP], BF16, tag=f"o1p{n}", name=f"o1p{n}") for n in range(NLOC)]
        y1_sb = [persist.tile([C, H * W], BF16, tag=f"y1_{n}", name=f"y1_{n}") for n in range(NLOC)]
        y2_sb = [persist.tile([C, H * W], BF16, tag=f"y2_{n}", name=f"y2_{n}") for n in range(NLOC)]

        w1_sb = persist.tile([C, 9 * C], BF16, tag="w1")
        w2_sb = persist.tile([C, 9 * C], BF16, tag="w2")
        nc.sync.dma_start(w1_sb[:], w1_d[:])
        # image 0 in two chunks so conv1 group 0 starts after the first
        x0v = xpad_sb[0][:].rearrange("p (h w) -> p h w", w=WP)
        nc.sync.dma_start(x0v[:, 0:26, :], xpad_d[0][:, 0:26, :])
        nc.sync.dma_start(x0v[:, 26:HP, :], xpad_d[0][:, 26:HP, :])
        nc.sync.dma_start(w2_sb[:], w2_d[:])
        for n in range(1, NLOC):
            nc.sync.dma_start(xpad_sb[n][:], xpad_d[n].rearrange("c h w -> c (h w)"))

        gam1 = persist.tile([C, 1], F32, tag="gam1")
        bet1 = persist.tile([C, 1], F32, tag="bet1")
        gam2 = persist.tile([C, 1], F32, tag="gam2")
        bet2 = persist.tile([C, 1], F32, tag="bet2")
        for t_sb, t_d in ((gam1, g1_d), (bet1, b1_d), (gam2, g2_d), (bet2, b2_d)):
            nc.sync.dma_start(t_sb[:], t_d.rearrange("(c one) -> c one", one=1))
        eps_sb = persist.tile([C, 1], F32, tag="eps")
        nc.vector.memset(eps_sb[:], EPS)

        for n in range(NLOC):
            # zero the halo of the conv2 input (interior is written by BN1)
            ov = o1p_sb[n][:].rearrange("p (h w) -> p h w", w=WP)
            nc.vector.memset(ov[:, 0, :], 0.0)
            nc.vector.memset(ov[:, HP - 1, :], 0.0)
            nc.vector.memset(ov[:, 1:HP - 1, 0:1], 0.0)
            nc.vector.memset(ov[:, 1:HP - 1, WP - 1:WP], 0.0)

        bnst1 = persist.tile([C, NLOC * NG, 6], F32, tag="bnst1")
        bnst2 = persist.tile([C, NLOC * NG, 6], F32, tag="bnst2")

        # ---- conv1 + stats ----
        _conv_phase(tc, w1_sb, xpad_sb, y1_sb, bnst1)
        a1, b1 = _bn_coeffs(tc, (small, dram), bnst1, gam1, bet1, eps_sb, 1,
                            fake_cc=fake_cc)

        # ---- out1 = relu(a1*y1 + b1), written into padded conv2 input.
        # Image 0 is split so conv2's first groups start sooner. ----
        for n in range(NLOC):
            ov = o1p_sb[n][:].rearrange("p (h w) -> p h w", w=WP)
            yv = y1_sb[n][:].rearrange("p (h w) -> p h w", w=W)
            splits = ((0, 16), (16, H)) if n == 0 else ((0, H),)
            for lo, hi in splits:
                nc.scalar.activation(ov[:, lo + 1:hi + 1, 1:W + 1],
                                     yv[:, lo:hi, :], act.Relu,
                                     bias=b1[:], scale=a1[:])

        # ---- conv2 + stats ----
        _conv_phase(tc, w2_sb, o1p_sb, y2_sb, bnst2)
        a2, b2 = _bn_coeffs(tc, (small, dram), bnst2, gam2, bet2, eps_sb, 2,
                            fake_cc=fake_cc)

        # ---- out = relu(a2*y2 + b2 + x) ----
        for n in range(NLOC):
            xv = xpad_sb[n][:].rearrange("p (h w) -> p h w", w=WP)
            yv = y2_sb[n][:].rearrange("p (h w) -> p h w", w=W)
            for g in range(NG):
                r0 = g * R
                s = fin.tile([C, R, W], F32, tag="s")
                nc.vector.scalar_tensor_tensor(
                    s[:], yv[:, r0:r0 + R, :], a2[:],
                    xv[:, r0 + 1:r0 + R + 1, 1:W + 1],
                    op0=alu.mult, op1=alu.add)
                # out = max(s + b2, 0); alternate between the GpSimd and
                # Scalar engines (both idle in the tail) to halve the span
                ob = ostage.tile([C, R, W], F32, tag="ob")
                if (n * NG + g) % 2 == 0:
                    nc.gpsimd.tensor_scalar(ob[:], s[:], b2[:], 0.0,
                                            op0=alu.add, op1=alu.max)
                else:
                    nc.scalar.activation(ob[:], s[:], act.Relu,
                                         bias=b2[:], scale=1.0)
                nc.sync.dma_start(out_d[n, :, r0:r0 + R, :], ob[:])


def _build_program(reps=1, fake_cc=False):
    key = ("nc", reps, fake_cc)
    if key in _CACHE:
        return _CACHE[key]
    nc = bacc.Bacc("TRN2", debug=False, num_devices=NCORES)
    xpad_d = nc.dram_tensor("xpad", [NLOC, C, HP, WP], BF16, kind="ExternalInput").ap()
    w1_d = nc.dram_tensor("w1t", [C, 9 * C], BF16, kind="ExternalInput").ap()
    w2_d = nc.dram_tensor("w2t", [C, 9 * C], BF16, kind="ExternalInput").ap()
    g1_d = nc.dram_tensor("gamma1", [C], F32, kind="ExternalInput").ap()
    b1_d = nc.dram_tensor("beta1", [C], F32, kind="ExternalInput").ap()
    g2_d = nc.dram_tensor("gamma2", [C], F32, kind="ExternalInput").ap()
    b2_d = nc.dram_tensor("beta2", [C], F32, kind="ExternalInput").ap()
    out_d = nc.dram_tensor("out", [NLOC, C, H, W], F32, kind="ExternalOutput").ap()

    with tile.TileContext(nc) as tc:
        _build_body(tc, xpad_d, w1_d, w2_d, g1_d, b1_d, g2_d, b2_d, out_d,
                    reps=reps, fake_cc=fake_cc)
    nc.compile()
    _CACHE[key] = nc
    return nc


def _prep_in_maps(inputs):
    x = np.asarray(inputs["x"], dtype=np.float32)
    w1 = np.asarray(inputs["w1"], dtype=np.float32)
    w2 = np.asarray(inputs["w2"], dtype=np.float32)

    def wprep(w):
        wb = np.sign(w).astype(np.float32)
        return np.ascontiguousarray(
            wb.transpose(1, 2, 3, 0).reshape(C, 9 * C)).astype(NP_BF16)

    xpad = np.zeros((N, C, HP, WP), dtype=NP_BF16)
    xpad[:, :, 1:H + 1, 1:W + 1] = x.astype(NP_BF16)

    common = {
        "w1t": wprep(w1),
        "w2t": wprep(w2),
        "gamma1": np.asarray(inputs["gamma1"], np.float32),
        "beta1": np.asarray(inputs["beta1"], np.float32),
        "gamma2": np.asarray(inputs["gamma2"], np.float32),
        "beta2": np.asarray(inputs["beta2"], np.float32),
    }
    return [
        {"xpad": np.ascontiguousarray(xpad[k * NLOC:(k + 1) * NLOC]), **common}
        for k in range(NCORES)
    ]


def _run(inputs, trace=False, trace_kwargs=None, reps=1):
    in_maps = _prep_in_maps(inputs)
    nc = _build_program(reps=reps)
    res = run_bass_kernel_spmd(
        nc, in_maps, core_ids=list(range(NCORES)), trace=trace,
        **(trace_kwargs or {}))
    out = np.concatenate([res.results[k]["out"] for k in range(NCORES)], axis=0)
    return out, res


def kernel(**inputs) -> np.ndarray:
    out, _ = _run(inputs, trace=False)
    return out



# revision 14
# speedup vs baseline: 10.0684x; 10.0684x over previous
"""Trainium2 Bass kernel for a binarized-conv ResNet BasicBlock.

    y1 = conv3x3(x, sign(w1)); out1 = relu(BN(y1))
    y2 = conv3x3(out1, sign(w2)); out = relu(BN(y2) + x)

BN is training-mode (batch stats over N,H,W). Sharding: data-parallel over
the batch (N=32 -> 4 images per core on 8 cores); conv weights + BN params
replicated; BN stats sync'd with a tiny [128,2] collective (sum, sumsq).

Conv mapping: C_in=128 lands exactly on the 128 SBUF partitions; a 3x3
conv is 9 shifted matmuls accumulated in PSUM (lhsT = [Cin, Cout] per tap,
rhs = padded input rows). Matmul inputs are bf16 (weights are exactly
+/-1); accumulation is fp32 in PSUM.  Padded width is 60 with the interior
at columns 2..57 so every row of the interior is 4-byte aligned (keeps the
DVE/ACT 2x packed modes on the elementwise passes).  The kernel output is
bf16 (upcast to fp32 on the host) to halve the store traffic.
"""

import numpy as np

import concourse.bass as bass
import concourse.tile as tile
from concourse import bacc, mybir
from concourse.bass_utils import run_bass_kernel_spmd

F32 = mybir.dt.float32
BF16 = mybir.dt.bfloat16
NP_BF16 = mybir.dt.np(BF16)

N, C, H, W = 32, 128, 56, 56
NCORES = 8
NLOC = N // NCORES           # images per core
HP, WP = H + 2, H + 4        # padded spatial dims; interior at [1:57, 2:58]
XOFF = 2                     # interior column offset (4B alignment)
R = 8                        # output rows per matmul group
NG = H // R                  # groups per image
F = R * W                    # moving free dim per matmul (448 <= 512)
CNT_GLB = N * H * W          # global BN count
EPS = 1e-5
WARM0 = 16                   # warmup junk matmuls before conv1
WARM1 = 52                   # keep-warm junk matmuls during the BN1 sync

_CACHE = {}


def _conv_phase(tc, w_sb, src_pads, dst_ys, bnst, gb=2):
    """One conv layer: 9-tap matmul accumulation per (image, row-group),
    PSUM evicted to SBUF bf16 via ACT copy, then bn_stats on the evicted
    tile for the sync-BN statistics."""
    nc = tc.nc
    groups = [(n, g) for n in range(NLOC) for g in range(NG)]
    xvs = [src_pads[n][:].rearrange("p (h w) -> p h w", w=WP) for n in range(NLOC)]
    with tc.tile_pool(name="psum", bufs=6, space="PSUM") as psum:
        for b0 in range(0, len(groups), gb):
            batch = groups[b0:b0 + gb]
            tiles = [psum.tile([C, F], F32, tag="ps", name=f"ps{b0 + i}")
                     for i in range(len(batch))]
            t = 0
            for ky in range(3):
                for kx in range(3):
                    for i, (n, g) in enumerate(batch):
                        r0 = g * R
                        nc.tensor.matmul(
                            tiles[i][:],
                            w_sb[:, t * C:(t + 1) * C],
                            xvs[n][:, r0 + ky:r0 + ky + R, kx + 1:kx + 1 + W],
                            start=(t == 0),
                            stop=(t == 8),
                        )
                    t += 1
            for i, (n, g) in enumerate(batch):
                r0 = g * R
                seg = dst_ys[n][:][:, r0 * W:(r0 + R) * W]
                nc.scalar.copy(seg, tiles[i][:])
                nc.vector.bn_stats(bnst[:, n * NG + g, :], seg)


def _junk_mms(tc, junk_in, junk_ps, count):
    """Dependency-free matmuls that keep the PE HAM-warm while it would
    otherwise idle (initial DMA wait, sync-BN collective)."""
    nc = tc.nc
    for _ in range(count):
        nc.tensor.matmul(junk_ps[:], junk_in[:, 0:C], junk_in[:, 0:F],
                         start=True, stop=True, skip_group_check=True)


def _bn_coeffs(tc, pools, bnst, gamma_sb, beta_sb, eps_sb, idx, fake_cc=False):
    """Local (mean,var) -> (sum,sumsq) partials, AllReduce across the 8
    cores, then a = gamma*rsqrt(var+eps), b = beta - mean*a.  All [128,1].

    fake_cc replaces the collective with a DRAM->DRAM copy so the program
    can run under the single-core TimelineSim (timing studies only)."""
    nc = tc.nc
    small, dram = pools
    alu = mybir.AluOpType
    act = mybir.ActivationFunctionType

    mv = small.tile([C, 2], F32, tag=f"mv{idx}")
    nc.vector.bn_aggr(mv[:], bnst[:])
    # partial sums: sum = mean*cnt ; sumsq = (var + mean^2)*cnt
    cnt_loc = float(NLOC * H * W)
    e2 = small.tile([C, 1], F32, tag=f"e2{idx}")
    nc.vector.scalar_tensor_tensor(
        e2[:], mv[:, 0:1], mv[:, 0:1], mv[:, 1:2], op0=alu.mult, op1=alu.add)
    part = small.tile([C, 2], F32, tag=f"part{idx}")
    nc.vector.tensor_scalar_mul(part[:, 0:1], mv[:, 0:1], cnt_loc)
    nc.vector.tensor_scalar_mul(part[:, 1:2], e2[:], cnt_loc)

    cc_in = dram.tile([C, 2], F32, tag=f"ccin{idx}")
    cc_out = dram.tile([C, 2], F32, tag=f"ccout{idx}")
    nc.sync.dma_start(cc_in[:], part[:])
    if fake_cc:
        nc.sync.dma_start(cc_out[:], cc_in[:])
    else:
        nc.gpsimd.collective_compute(
            "AllReduce",
            alu.add,
            replica_groups=[list(range(NCORES))],
            ins=[cc_in[:].opt()],
            outs=[cc_out[:].opt()],
        )
    gl = small.tile([C, 2], F32, tag=f"gl{idx}")
    nc.sync.dma_start(gl[:], cc_out[:])

    # global (sum, sumsq) -> (mean, E[y^2]) in one pass
    gm = small.tile([C, 2], F32, tag=f"gm{idx}")
    nc.vector.tensor_scalar_mul(gm[:], gl[:], 1.0 / float(CNT_GLB))
    mg = gm[:, 0:1]
    # negvar = mean^2 - E[y^2]; std = sqrt(-negvar + eps); inv = 1/std
    negvar = small.tile([C, 1], F32, tag=f"negvar{idx}")
    nc.vector.scalar_tensor_tensor(
        negvar[:], mg, mg, gm[:, 1:2], op0=alu.mult, op1=alu.subtract)
    std = small.tile([C, 1], F32, tag=f"std{idx}")
    nc.scalar.activation(std[:], negvar[:], act.Sqrt,
                         bias=eps_sb[:], scale=-1.0)
    inv = small.tile([C, 1], F32, tag=f"inv{idx}")
    nc.vector.reciprocal(inv[:], std[:])
    a_t = small.tile([C, 1], F32, tag=f"a{idx}")
    nc.vector.tensor_mul(a_t[:], gamma_sb[:], inv[:])
    ma = small.tile([C, 1], F32, tag=f"ma{idx}")
    nc.vector.tensor_mul(ma[:], mg, a_t[:])
    b_t = small.tile([C, 1], F32, tag=f"b{idx}")
    nc.vector.tensor_tensor(b_t[:], beta_sb[:], ma[:], op=alu.subtract)
    return a_t, b_t


def _build_body(tc, xpad_d, w1_d, w2_d, g1_d, b1_d, g2_d, b2_d, out_d,
                reps=1, fake_cc=False):
    nc = tc.nc

    with (
        tc.tile_pool(name="persist", bufs=1) as persist,
        tc.tile_pool(name="small", bufs=1) as small,
        tc.tile_pool(name="dram", bufs=1, space="DRAM") as dram,
        tc.tile_pool(name="fin", bufs=4) as fin,
        tc.tile_pool(name="ostage", bufs=4) as ostage,
        tc.tile_pool(name="psumj", bufs=1, space="PSUM") as psumj,
    ):
        pools = (persist, small, dram, fin, ostage, psumj)
        args = (xpad_d, w1_d, w2_d, g1_d, b1_d, g2_d, b2_d, out_d)
        if reps == 1:
            _emit_iteration(tc, pools, args, fake_cc)
        else:
            with tc.For_i(0, reps, 1):
                _emit_iteration(tc, pools, args, fake_cc)


def _emit_iteration(tc, pools, args, fake_cc):
    nc = tc.nc
    act = mybir.ActivationFunctionType
    alu = mybir.AluOpType
    persist, small, dram, fin, ostage, psumj = pools
    xpad_d, w1_d, w2_d, g1_d, b1_d, g2_d, b2_d, out_d = args
    if True:
        # ---- per-image persistent buffers (x load issued first: the first
        # conv group waits on image 0) ----
        xpad_sb = [persist.tile([C, HP * WP], BF16, tag=f"xp{n}", name=f"xp{n}") for n in range(NLOC)]
        o1p_sb = [persist.tile([C, HP * WP], BF16, tag=f"o1p{n}", name=f"o1p{n}") for n in range(NLOC)]
        y1_sb = [persist.tile([C, H * W], BF16, tag=f"y1_{n}", name=f"y1_{n}") for n in range(NLOC)]
        y2_sb = [persist.tile([C, H * W], BF16, tag=f"y2_{n}", name=f"y2_{n}") for n in range(NLOC)]

        w1_sb = persist.tile([C, 9 * C], BF16, tag="w1")
        w2_sb = persist.tile([C, 9 * C], BF16, tag="w2")
        nc.sync.dma_start(w1_sb[:], w1_d[:])
        # image 0 in three chunks so conv1 group 0 starts after the first
        x0v = xpad_sb[0][:].rearrange("p (h w) -> p h w", w=WP)
        nc.sync.dma_start(x0v[:, 0:10, :], xpad_d[0][:, 0:10, :])
        nc.sync.dma_start(x0v[:, 10:26, :], xpad_d[0][:, 10:26, :])
        nc.sync.dma_start(x0v[:, 26:HP, :], xpad_d[0][:, 26:HP, :])
        nc.sync.dma_start(w2_sb[:], w2_d[:])
        for n in range(1, NLOC):
            nc.sync.dma_start(xpad_sb[n][:], xpad_d[n].rearrange("c h w -> c (h w)"))

        gam1 = persist.tile([C, 1], F32, tag="gam1")
        bet1 = persist.tile([C, 1], F32, tag="bet1")
        gam2 = persist.tile([C, 1], F32, tag="gam2")
        bet2 = persist.tile([C, 1], F32, tag="bet2")
        for t_sb, t_d in ((gam1, g1_d), (bet1, b1_d), (gam2, g2_d), (bet2, b2_d)):
            nc.sync.dma_start(t_sb[:], t_d.rearrange("(c one) -> c one", one=1))
        eps_sb = persist.tile([C, 1], F32, tag="eps")
        nc.vector.memset(eps_sb[:], EPS)

        # ---- ACT table preload (off the critical path): Rsqrt anchors the
        # set used later by the BN coeff chain; Relu/Copy are set fillers.
        tl0 = small.tile([C, 1], F32, tag="tl0")
        nc.scalar.activation(tl0[:], eps_sb[:], act.Sqrt)
        nc.scalar.activation(tl0[:], eps_sb[:], act.Relu)

        # ---- PE warmup: junk matmuls with no dependencies run while the
        # first image chunks stream in, so conv1 starts HAM-warm.
        junk_in = persist.tile([C, F], BF16, tag="junk")
        nc.vector.memset(junk_in[:], 0.0)
        junk_ps = psumj.tile([C, F], F32, tag="junkps")
        _junk_mms(tc, junk_in, junk_ps, WARM0)

        for n in range(NLOC):
            # zero the halo of the conv2 input (interior is written by BN1)
            ov = o1p_sb[n][:].rearrange("p (h w) -> p h w", w=WP)
            nc.vector.memset(ov[:, 0, :], 0.0)
            nc.vector.memset(ov[:, HP - 1, :], 0.0)
            nc.vector.memset(ov[:, 1:HP - 1, 0:XOFF], 0.0)
            nc.vector.memset(ov[:, 1:HP - 1, XOFF + W:WP], 0.0)

        bnst1 = persist.tile([C, NLOC * NG, 6], F32, tag="bnst1")
        bnst2 = persist.tile([C, NLOC * NG, 6], F32, tag="bnst2")

        # ---- conv1 + stats ----
        _conv_phase(tc, w1_sb, xpad_sb, y1_sb, bnst1)
        # keep the PE warm through the sync-BN gap (these have no deps and
        # drain right after conv1's last matmul, well before AR completes)
        _junk_mms(tc, junk_in, junk_ps, WARM1)
        a1, b1 = _bn_coeffs(tc, (small, dram), bnst1, gam1, bet1, eps_sb, 1,
                            fake_cc=fake_cc)

        # ---- out1 = relu(a1*y1 + b1), written into padded conv2 input.
        # Image 0 is split so conv2's first batch (groups 0-1) starts as
        # soon as rows 1..18 are in place. ----
        for n in range(NLOC):
            ov = o1p_sb[n][:].rearrange("p (h w) -> p h w", w=WP)
            yv = y1_sb[n][:].rearrange("p (h w) -> p h w", w=W)
            splits = ((0, 16), (16, H)) if n == 0 else ((0, H),)
            for lo, hi in splits:
                nc.scalar.activation(ov[:, lo + 1:hi + 1, XOFF:XOFF + W],
                                     yv[:, lo:hi, :], act.Relu,
                                     bias=b1[:], scale=a1[:])

        # ---- conv2 + stats ----
        _conv_phase(tc, w2_sb, o1p_sb, y2_sb, bnst2)
        a2, b2 = _bn_coeffs(tc, (small, dram), bnst2, gam2, bet2, eps_sb, 2,
                            fake_cc=fake_cc)

        # ---- out = relu(a2*y2 + b2 + x), bf16, in half-image chunks:
        # u = a2*y2 + b2 runs as tensor_scalar (4x on DVE); the two-tensor
        # +x add and the relu are spread across DVE/Pool/ACT to balance the
        # tail; the store DMA streams out per chunk. ----
        HC = H // 2
        chunks = [(n, h) for n in range(NLOC) for h in range(2)]
        for i, (n, h) in enumerate(chunks):
            xv = xpad_sb[n][:].rearrange("p (h w) -> p h w", w=WP)
            yv = y2_sb[n][:].rearrange("p (h w) -> p h w", w=W)
            r0 = h * HC
            u = fin.tile([C, HC, W], BF16, tag="u")
            nc.vector.tensor_scalar(u[:], yv[:, r0:r0 + HC, :], a2[:], b2[:],
                                    op0=alu.mult, op1=alu.add)
            v = fin.tile([C, HC, W], BF16, tag="v")
            xs = xv[:, r0 + 1:r0 + HC + 1, XOFF:XOFF + W]
            if i % 4 == 3:
                nc.gpsimd.tensor_tensor(v[:], u[:], xs, op=alu.add)
            else:
                nc.vector.tensor_tensor(v[:], u[:], xs, op=alu.add)
            ob = ostage.tile([C, HC, W], BF16, tag="ob")
            if i < 6:
                nc.scalar.activation(ob[:], v[:], act.Relu)
            else:
                nc.vector.tensor_scalar(ob[:], v[:], 1.0, 0.0,
                                        op0=alu.mult, op1=alu.max)
            nc.sync.dma_start(out_d[n, :, r0:r0 + HC, :], ob[:])


def _build_program(reps=1, fake_cc=False):
    key = ("nc", reps, fake_cc)
    if key in _CACHE:
        return _CACHE[key]
    nc = bacc.Bacc("TRN2", debug=False, num_devices=NCORES)
    xpad_d = nc.dram_tensor("xpad", [NLOC, C, HP, WP], BF16, kind="ExternalInput").ap()
    w1_d = nc.dram_tensor("w1t", [C, 9 * C], BF16, kind="ExternalInput").ap()
    w2_d = nc.dram_tensor("w2t", [C, 9 * C], BF16, kind="ExternalInput").ap()
    g1_d = nc.dram_tensor("gamma1", [C], F32, kind="ExternalInput").ap()
    b1_d = nc.dram_tensor("beta1", [C], F32, kind="ExternalInput").ap()
    g2_d = nc.dram_tensor("gamma2", [C], F32, kind="ExternalInput").ap()
    b2_d = nc.dram_tensor("beta2", [C], F32, kind="ExternalInput").ap()
    out_d = nc.dram_tensor("out", [NLOC, C, H, W], BF16, kind="ExternalOutput").ap()

    with tile.TileContext(nc) as tc:
        _build_body(tc, xpad_d, w1_d, w2_d, g1_d, b1_d, g2_d, b2_d, out_d,
                    reps=reps, fake_cc=fake_cc)
    nc.compile()
    _CACHE[key] = nc
    return nc


def _prep_in_maps(inputs):
    x = np.asarray(inputs["x"], dtype=np.float32)
    w1 = np.asarray(inputs["w1"], dtype=np.float32)
    w2 = np.asarray(inputs["w2"], dtype=np.float32)

    def wprep(w):
        wb = np.sign(w).astype(np.float32)
        return np.ascontiguousarray(
            wb.transpose(1, 2, 3, 0).reshape(C, 9 * C)).astype(NP_BF16)

    xpad = np.zeros((N, C, HP, WP), dtype=NP_BF16)
    xpad[:, :, 1:H + 1, XOFF:XOFF + W] = x.astype(NP_BF16)

    common = {
        "w1t": wprep(w1),
        "w2t": wprep(w2),
        "gamma1": np.asarray(inputs["gamma1"], np.float32),
        "beta1": np.asarray(inputs["beta1"], np.float32),
        "gamma2": np.asarray(inputs["gamma2"], np.float32),
        "beta2": np.asarray(inputs["beta2"], np.float32),
    }
    return [
        {"xpad": np.ascontiguousarray(xpad[k * NLOC:(k + 1) * NLOC]), **common}
        for k in range(NCORES)
    ]


def _run(inputs, trace=False, trace_kwargs=None, reps=1):
    in_maps = _prep_in_maps(inputs)
    nc = _build_program(reps=reps)
    res = run_bass_kernel_spmd(
        nc, in_maps, core_ids=list(range(NCORES)), trace=trace,
        **(trace_kwargs or {}))
    out = np.concatenate(
        [res.results[k]["out"].astype(np.float32) for k in range(NCORES)],
        axis=0)
    return out, res


def kernel(**inputs) -> np.ndarray:
    out, _ = _run(inputs, trace=False)
    return out


# revision 27
# speedup vs baseline: 13.0841x; 1.2995x over previous
"""Trainium2 Bass kernel for a binarized-conv ResNet BasicBlock.

    y1 = conv3x3(x, sign(w1)); out1 = relu(BN(y1))
    y2 = conv3x3(out1, sign(w2)); out = relu(BN(y2) + x)

BN is training-mode (batch stats over N,H,W). Sharding: data-parallel over
the batch (N=32 -> 4 images per core on 8 cores); conv weights + BN params
replicated; BN stats sync'd with a tiny [128,2] collective (sum, sumsq).

Conv mapping: C_in=128 lands exactly on the 128 SBUF partitions; a 3x3
conv is 9 shifted matmuls accumulated in PSUM (lhsT = [Cin, Cout] per tap,
rhs = padded input rows). Matmul inputs are bf16 (weights are exactly
+/-1); accumulation is fp32 in PSUM.  Padded width is 60 with the interior
at columns 2..57 so every row of the interior is 4-byte aligned (keeps the
DVE/ACT 2x packed modes on the elementwise passes).  The kernel output is
bf16 (upcast to fp32 on the host) to halve the store traffic.
"""

import numpy as np

import concourse.bass as bass
import concourse.tile as tile
from concourse import bacc, mybir
from concourse.bass_utils import run_bass_kernel_spmd

F32 = mybir.dt.float32
BF16 = mybir.dt.bfloat16
NP_BF16 = mybir.dt.np(BF16)

N, C, H, W = 32, 128, 56, 56
NCORES = 8
NLOC = N // NCORES           # images per core
HP, WP = H + 2, H + 4        # padded spatial dims; interior at [1:57, 2:58]
XOFF = 2                     # interior column offset (4B alignment)
R = 8                        # output rows per matmul group
NG = H // R                  # groups per image
F = R * W                    # moving free dim per matmul (448 <= 512)
CNT_GLB = N * H * W          # global BN count
EPS = 1e-5
WARM0 = 16                   # warmup junk matmuls before conv1
WARM1 = 52                   # keep-warm junk matmuls during the BN1 sync

_CACHE = {}


def _conv_phase(tc, w_sb, src_pads, dst_ys, bnst, gb=2):
    """One conv layer: 9-tap matmul accumulation per (image, row-group),
    PSUM evicted to SBUF bf16 via ACT copy, then bn_stats on the evicted
    tile for the sync-BN statistics."""
    nc = tc.nc
    groups = [(n, g) for n in range(NLOC) for g in range(NG)]
    xvs = [src_pads[n][:].rearrange("p (h w) -> p h w", w=WP) for n in range(NLOC)]
    with tc.tile_pool(name="psum", bufs=6, space="PSUM") as psum:
        for b0 in range(0, len(groups), gb):
            batch = groups[b0:b0 + gb]
            tiles = [psum.tile([C, F], F32, tag="ps", name=f"ps{b0 + i}")
                     for i in range(len(batch))]
            t = 0
            for ky in range(3):
                for kx in range(3):
                    for i, (n, g) in enumerate(batch):
                        r0 = g * R
                        nc.tensor.matmul(
                            tiles[i][:],
                            w_sb[:, t * C:(t + 1) * C],
                            xvs[n][:, r0 + ky:r0 + ky + R, kx + 1:kx + 1 + W],
                            start=(t == 0),
                            stop=(t == 8),
                        )
                    t += 1
            for i, (n, g) in enumerate(batch):
                r0 = g * R
                seg = dst_ys[n][:][:, r0 * W:(r0 + R) * W]
                nc.scalar.copy(seg, tiles[i][:])
                nc.vector.bn_stats(bnst[:, n * NG + g, :], seg)


FJ = 448                     # junk matmul free dim


def _junk_mms(tc, junk_in, junk_ps, count):
    """Dependency-free matmuls that keep the PE HAM-warm while it would
    otherwise idle (initial DMA wait, sync-BN collective)."""
    nc = tc.nc
    for k in range(count):
        nc.tensor.matmul(junk_ps[0][:], junk_in[:, 0:C],
                         junk_in[:, 0:FJ],
                         start=True, stop=True, skip_group_check=True)


def _bn_coeffs(tc, pools, bnst, gamma_sb, beta_sb, eps_sb, idx, fake_cc=False):
    """Local (mean,var) -> (sum,sumsq) partials, AllReduce across the 8
    cores, then a = gamma*rsqrt(var+eps), b = beta - mean*a.  All [128,1].

    fake_cc replaces the collective with a DRAM->DRAM copy so the program
    can run under the single-core TimelineSim (timing studies only)."""
    nc = tc.nc
    small, dram = pools
    alu = mybir.AluOpType
    act = mybir.ActivationFunctionType

    mv = small.tile([C, 2], F32, tag=f"mv{idx}")
    nc.vector.bn_aggr(mv[:], bnst[:])
    # partial sums: sum = mean*cnt ; sumsq = (var + mean^2)*cnt
    cnt_loc = float(NLOC * H * W)
    e2 = small.tile([C, 1], F32, tag=f"e2{idx}")
    nc.vector.scalar_tensor_tensor(
        e2[:], mv[:, 0:1], mv[:, 0:1], mv[:, 1:2], op0=alu.mult, op1=alu.add)
    part = small.tile([C, 2], F32, tag=f"part{idx}")
    nc.vector.tensor_scalar_mul(part[:, 0:1], mv[:, 0:1], cnt_loc)
    nc.vector.tensor_scalar_mul(part[:, 1:2], e2[:], cnt_loc)

    cc_in = dram.tile([C, 2], F32, tag=f"ccin{idx}")
    cc_out = dram.tile([C, 2], F32, tag=f"ccout{idx}")
    nc.sync.dma_start(cc_in[:], part[:])
    if fake_cc:
        nc.sync.dma_start(cc_out[:], cc_in[:])
    else:
        nc.gpsimd.collective_compute(
            "AllReduce",
            alu.add,
            replica_groups=[list(range(NCORES))],
            ins=[cc_in[:].opt()],
            outs=[cc_out[:].opt()],
        )
    gl = small.tile([C, 2], F32, tag=f"gl{idx}")
    nc.sync.dma_start(gl[:], cc_out[:])

    # global (sum, sumsq) -> (mean, E[y^2]) in one pass
    gm = small.tile([C, 2], F32, tag=f"gm{idx}")
    nc.vector.tensor_scalar_mul(gm[:], gl[:], 1.0 / float(CNT_GLB))
    mg = gm[:, 0:1]
    # negvar = mean^2 - E[y^2]; std = sqrt(-negvar + eps); inv = 1/std
    negvar = small.tile([C, 1], F32, tag=f"negvar{idx}")
    nc.vector.scalar_tensor_tensor(
        negvar[:], mg, mg, gm[:, 1:2], op0=alu.mult, op1=alu.subtract)
    std = small.tile([C, 1], F32, tag=f"std{idx}")
    nc.scalar.activation(std[:], negvar[:], act.Sqrt,
                         bias=eps_sb[:], scale=-1.0)
    inv = small.tile([C, 1], F32, tag=f"inv{idx}")
    nc.vector.reciprocal(inv[:], std[:])
    a_t = small.tile([C, 1], F32, tag=f"a{idx}")
    nc.vector.tensor_mul(a_t[:], gamma_sb[:], inv[:])
    ma = small.tile([C, 1], F32, tag=f"ma{idx}")
    nc.vector.tensor_mul(ma[:], mg, a_t[:])
    b_t = small.tile([C, 1], F32, tag=f"b{idx}")
    nc.vector.tensor_tensor(b_t[:], beta_sb[:], ma[:], op=alu.subtract)
    return a_t, b_t


def _build_body(tc, xpad_d, w1_d, w2_d, g1_d, b1_d, g2_d, b2_d, out_d,
                reps=1, fake_cc=False):
    nc = tc.nc

    with (
        tc.tile_pool(name="persist", bufs=1) as persist,
        tc.tile_pool(name="small", bufs=1) as small,
        tc.tile_pool(name="dram", bufs=1, space="DRAM") as dram,
        tc.tile_pool(name="fin", bufs=4) as fin,
        tc.tile_pool(name="ostage", bufs=4) as ostage,
        tc.tile_pool(name="psumj", bufs=1, space="PSUM") as psumj,
    ):
        pools = (persist, small, dram, fin, ostage, psumj)
        args = (xpad_d, w1_d, w2_d, g1_d, b1_d, g2_d, b2_d, out_d)
        if reps == 1:
            _emit_iteration(tc, pools, args, fake_cc)
        else:
            with tc.For_i(0, reps, 1):
                _emit_iteration(tc, pools, args, fake_cc)


def _emit_iteration(tc, pools, args, fake_cc):
    nc = tc.nc
    act = mybir.ActivationFunctionType
    alu = mybir.AluOpType
    persist, small, dram, fin, ostage, psumj = pools
    xpad_d, w1_d, w2_d, g1_d, b1_d, g2_d, b2_d, out_d = args
    if True:
        # ---- per-image persistent buffers (x load issued first: the first
        # conv group waits on image 0) ----
        xpad_sb = [persist.tile([C, HP * WP], BF16, tag=f"xp{n}", name=f"xp{n}") for n in range(NLOC)]
        o1p_sb = [persist.tile([C, HP * WP], BF16, tag=f"o1p{n}", name=f"o1p{n}") for n in range(NLOC)]
        y1_sb = [persist.tile([C, H * W], BF16, tag=f"y1_{n}", name=f"y1_{n}") for n in range(NLOC)]
        y2_sb = [persist.tile([C, H * W], BF16, tag=f"y2_{n}", name=f"y2_{n}") for n in range(NLOC)]

        w1_sb = persist.tile([C, 9 * C], BF16, tag="w1")
        w2_sb = persist.tile([C, 9 * C], BF16, tag="w2")
        nc.sync.dma_start(w1_sb[:], w1_d[:])
        # image 0 in three chunks so conv1 group 0 starts after the first
        x0v = xpad_sb[0][:].rearrange("p (h w) -> p h w", w=WP)
        nc.sync.dma_start(x0v[:, 0:10, :], xpad_d[0][:, 0:10, :])
        nc.sync.dma_start(x0v[:, 10:26, :], xpad_d[0][:, 10:26, :])
        nc.sync.dma_start(x0v[:, 26:HP, :], xpad_d[0][:, 26:HP, :])
        nc.sync.dma_start(w2_sb[:], w2_d[:])
        for n in range(1, NLOC):
            nc.sync.dma_start(xpad_sb[n][:], xpad_d[n].rearrange("c h w -> c (h w)"))

        gam1 = persist.tile([C, 1], F32, tag="gam1")
        bet1 = persist.tile([C, 1], F32, tag="bet1")
        gam2 = persist.tile([C, 1], F32, tag="gam2")
        bet2 = persist.tile([C, 1], F32, tag="bet2")
        for t_sb, t_d in ((gam1, g1_d), (bet1, b1_d), (gam2, g2_d), (bet2, b2_d)):
            nc.sync.dma_start(t_sb[:], t_d.rearrange("(c one) -> c one", one=1))
        eps_sb = persist.tile([C, 1], F32, tag="eps")
        nc.vector.memset(eps_sb[:], EPS)

        # ---- ACT table preload (off the critical path): Rsqrt anchors the
        # set used later by the BN coeff chain; Relu/Copy are set fillers.
        tl0 = small.tile([C, 1], F32, tag="tl0")
        nc.scalar.activation(tl0[:], eps_sb[:], act.Sqrt)
        nc.scalar.activation(tl0[:], eps_sb[:], act.Relu)

        # ---- PE warmup: junk matmuls with no dependencies run while the
        # first image chunks stream in, so conv1 starts HAM-warm.
        junk_in = persist.tile([C, FJ], BF16, tag="junk")
        nc.vector.memset(junk_in[:], 0.0)
        junk_ps = [psumj.tile([C, FJ], F32, tag="junkps", name="jp0")]
        _junk_mms(tc, junk_in, junk_ps, WARM0)

        for n in range(NLOC):
            # zero the halo of the conv2 input (interior is written by BN1)
            ov = o1p_sb[n][:].rearrange("p (h w) -> p h w", w=WP)
            nc.vector.memset(ov[:, 0, :], 0.0)
            nc.vector.memset(ov[:, HP - 1, :], 0.0)
            nc.vector.memset(ov[:, 1:HP - 1, 0:XOFF], 0.0)
            nc.vector.memset(ov[:, 1:HP - 1, XOFF + W:WP], 0.0)

        bnst1 = persist.tile([C, NLOC * NG, 6], F32, tag="bnst1")
        bnst2 = persist.tile([C, NLOC * NG, 6], F32, tag="bnst2")

        # ---- conv1 + stats ----
        _conv_phase(tc, w1_sb, xpad_sb, y1_sb, bnst1)
        # keep the PE warm through the sync-BN gap (these have no deps and
        # drain right after conv1's last matmul, well before AR completes)
        _junk_mms(tc, junk_in, junk_ps, WARM1)
        a1, b1 = _bn_coeffs(tc, (small, dram), bnst1, gam1, bet1, eps_sb, 1,
                            fake_cc=fake_cc)

        # ---- out1 = relu(a1*y1 + b1), written into padded conv2 input.
        # Image 0 is split so conv2's first batch (groups 0-1) starts as
        # soon as rows 1..18 are in place. ----
        for n in range(NLOC):
            ov = o1p_sb[n][:].rearrange("p (h w) -> p h w", w=WP)
            yv = y1_sb[n][:].rearrange("p (h w) -> p h w", w=W)
            splits = ((0, 16), (16, H)) if n == 0 else ((0, H),)
            for lo, hi in splits:
                nc.scalar.activation(ov[:, lo + 1:hi + 1, XOFF:XOFF + W],
                                     yv[:, lo:hi, :], act.Relu,
                                     bias=b1[:], scale=a1[:])

        # ---- conv2 + stats ----
        _conv_phase(tc, w2_sb, o1p_sb, y2_sb, bnst2)
        a2, b2 = _bn_coeffs(tc, (small, dram), bnst2, gam2, bet2, eps_sb, 2,
                            fake_cc=fake_cc)

        # ---- out = relu(a2*y2 + b2 + x), bf16, in half-image chunks:
        # u = a2*y2 + b2 runs as tensor_scalar (4x on DVE); the two-tensor
        # +x add and the relu are spread across DVE/Pool/ACT to balance the
        # tail; the store DMA streams out per chunk. ----
        HC = H // 2
        chunks = [(n, h) for n in range(NLOC) for h in range(2)]
        for i, (n, h) in enumerate(chunks):
            xv = xpad_sb[n][:].rearrange("p (h w) -> p h w", w=WP)
            yv = y2_sb[n][:].rearrange("p (h w) -> p h w", w=W)
            r0 = h * HC
            u = fin.tile([C, HC, W], BF16, tag="u")
            nc.vector.tensor_scalar(u[:], yv[:, r0:r0 + HC, :], a2[:], b2[:],
                                    op0=alu.mult, op1=alu.add)
            v = fin.tile([C, HC, W], BF16, tag="v")
            xs = xv[:, r0 + 1:r0 + HC + 1, XOFF:XOFF + W]
            if i % 4 == 3:
                nc.gpsimd.tensor_tensor(v[:], u[:], xs, op=alu.add)
            else:
                nc.vector.tensor_tensor(v[:], u[:], xs, op=alu.add)
            ob = ostage.tile([C, HC, W], BF16, tag="ob")
            if i < 6:
                nc.scalar.activation(ob[:], v[:], act.Relu)
            else:
                nc.vector.tensor_scalar(ob[:], v[:], 1.0, 0.0,
                                        op0=alu.mult, op1=alu.max)
            nc.sync.dma_start(out_d[n, :, r0:r0 + HC, :], ob[:])


def _build_program(reps=1, fake_cc=False, tiny_io=False):
    """tiny_io: timing-only variant -- the heavy tensors become Internal
    DRAM (contents garbage, sizes/DMAs identical) so the benchmark doesn't
    ship ~33MB over the axon tunnel per call.  A tiny real output keeps the
    program well-formed."""
    key = ("nc", reps, fake_cc, tiny_io)
    if key in _CACHE:
        return _CACHE[key]
    nc = bacc.Bacc("TRN2", debug=False, num_devices=NCORES)
    heavy = "Internal" if tiny_io else "ExternalInput"
    heavy_out = "Internal" if tiny_io else "ExternalOutput"
    xpad_d = nc.dram_tensor("xpad", [NLOC, C, HP, WP], BF16, kind=heavy).ap()
    w1_d = nc.dram_tensor("w1t", [C, 9 * C], BF16, kind=heavy).ap()
    w2_d = nc.dram_tensor("w2t", [C, 9 * C], BF16, kind=heavy).ap()
    g1_d = nc.dram_tensor("gamma1", [C], F32, kind="ExternalInput").ap()
    b1_d = nc.dram_tensor("beta1", [C], F32, kind="ExternalInput").ap()
    g2_d = nc.dram_tensor("gamma2", [C], F32, kind="ExternalInput").ap()
    b2_d = nc.dram_tensor("beta2", [C], F32, kind="ExternalInput").ap()
    out_d = nc.dram_tensor("out", [NLOC, C, H, W], BF16, kind=heavy_out).ap()
    if tiny_io:
        tout_d = nc.dram_tensor("tout", [C, 1], F32, kind="ExternalOutput").ap()

    with tile.TileContext(nc) as tc:
        _build_body(tc, xpad_d, w1_d, w2_d, g1_d, b1_d, g2_d, b2_d, out_d,
                    reps=reps, fake_cc=fake_cc)
    nc.compile()
    _CACHE[key] = nc
    return nc


def _prep_in_maps(inputs):
    x = np.asarray(inputs["x"], dtype=np.float32)
    w1 = np.asarray(inputs["w1"], dtype=np.float32)
    w2 = np.asarray(inputs["w2"], dtype=np.float32)

    def wprep(w):
        wb = np.sign(w).astype(np.float32)
        return np.ascontiguousarray(
            wb.transpose(1, 2, 3, 0).reshape(C, 9 * C)).astype(NP_BF16)

    xpad = np.zeros((N, C, HP, WP), dtype=NP_BF16)
    xpad[:, :, 1:H + 1, XOFF:XOFF + W] = x.astype(NP_BF16)

    common = {
        "w1t": wprep(w1),
        "w2t": wprep(w2),
        "gamma1": np.asarray(inputs["gamma1"], np.float32),
        "beta1": np.asarray(inputs["beta1"], np.float32),
        "gamma2": np.asarray(inputs["gamma2"], np.float32),
        "beta2": np.asarray(inputs["beta2"], np.float32),
    }
    return [
        {"xpad": np.ascontiguousarray(xpad[k * NLOC:(k + 1) * NLOC]), **common}
        for k in range(NCORES)
    ]


def _run(inputs, trace=False, trace_kwargs=None, reps=1):
    in_maps = _prep_in_maps(inputs)
    nc = _build_program(reps=reps)
    res = run_bass_kernel_spmd(
        nc, in_maps, core_ids=list(range(NCORES)), trace=trace,
        **(trace_kwargs or {}))
    out = np.concatenate(
        [res.results[k]["out"].astype(np.float32) for k in range(NCORES)],
        axis=0)
    return out, res


def kernel(**inputs) -> np.ndarray:
    out, _ = _run(inputs, trace=False)
    return out


# revision 31
# speedup vs baseline: 13.5355x; 1.0345x over previous
"""Trainium2 Bass kernel for a binarized-conv ResNet BasicBlock.

    y1 = conv3x3(x, sign(w1)); out1 = relu(BN(y1))
    y2 = conv3x3(out1, sign(w2)); out = relu(BN(y2) + x)

BN is training-mode (batch stats over N,H,W). Sharding: data-parallel over
the batch (N=32 -> 4 images per core on 8 cores); conv weights + BN params
replicated; BN stats sync'd with a tiny [128,2] collective (sum, sumsq).

Conv mapping: C_in=128 lands exactly on the 128 SBUF partitions; a 3x3
conv is 9 shifted matmuls accumulated in PSUM (lhsT = [Cin, Cout] per tap,
rhs = padded input rows). Matmul inputs are bf16 (weights are exactly
+/-1); accumulation is fp32 in PSUM.  Padded width is 60 with the interior
at columns 2..57 so every row of the interior is 4-byte aligned (keeps the
DVE/ACT 2x packed modes on the elementwise passes).  The kernel output is
bf16 (upcast to fp32 on the host) to halve the store traffic.
"""

import numpy as np

import concourse.bass as bass
import concourse.tile as tile
from concourse import bacc, mybir
from concourse.bass_utils import run_bass_kernel_spmd

F32 = mybir.dt.float32
BF16 = mybir.dt.bfloat16
NP_BF16 = mybir.dt.np(BF16)

N, C, H, W = 32, 128, 56, 56
NCORES = 8
NLOC = N // NCORES           # images per core
HP, WP = H + 2, H + 4        # padded spatial dims; interior at [1:57, 2:58]
XOFF = 2                     # interior column offset (4B alignment)
R = 8                        # output rows per matmul group
NG = H // R                  # groups per image
F = R * W                    # moving free dim per matmul (448 <= 512)
CNT_GLB = N * H * W          # global BN count
EPS = 1e-5
WARM0 = 16                   # warmup junk matmuls before conv1
WARM1 = 52                   # keep-warm junk matmuls during the BN1 sync

_CACHE = {}


def _conv_phase(tc, w_sb, src_pads, dst_ys, bnst, gb=2):
    """One conv layer: 9-tap matmul accumulation per (image, row-group),
    PSUM evicted to SBUF bf16 via ACT copy, then bn_stats on the evicted
    tile for the sync-BN statistics."""
    nc = tc.nc
    groups = [(n, g) for n in range(NLOC) for g in range(NG)]
    xvs = [src_pads[n][:].rearrange("p (h w) -> p h w", w=WP) for n in range(NLOC)]
    with tc.tile_pool(name="psum", bufs=6, space="PSUM") as psum:
        for b0 in range(0, len(groups), gb):
            batch = groups[b0:b0 + gb]
            tiles = [psum.tile([C, F], F32, tag="ps", name=f"ps{b0 + i}")
                     for i in range(len(batch))]
            t = 0
            for ky in range(3):
                for kx in range(3):
                    for i, (n, g) in enumerate(batch):
                        r0 = g * R
                        nc.tensor.matmul(
                            tiles[i][:],
                            w_sb[:, t * C:(t + 1) * C],
                            xvs[n][:, r0 + ky:r0 + ky + R, kx + 1:kx + 1 + W],
                            start=(t == 0),
                            stop=(t == 8),
                        )
                    t += 1
            for i, (n, g) in enumerate(batch):
                r0 = g * R
                seg = dst_ys[n][:][:, r0 * W:(r0 + R) * W]
                nc.scalar.copy(seg, tiles[i][:])
                nc.vector.bn_stats(bnst[:, n * NG + g, :], seg)


FJ = 448                     # junk matmul free dim


def _junk_mms(tc, junk_in, junk_ps, count):
    """Dependency-free matmuls that keep the PE HAM-warm while it would
    otherwise idle (initial DMA wait, sync-BN collective)."""
    nc = tc.nc
    for k in range(count):
        nc.tensor.matmul(junk_ps[0][:], junk_in[:, 0:C],
                         junk_in[:, 0:FJ],
                         start=True, stop=True, skip_group_check=True)


def _bn_coeffs(tc, pools, bnst, gamma_sb, beta_sb, eps_sb, idx, fake_cc=False):
    """Local (mean,var) -> (sum,sumsq) partials, AllReduce across the 8
    cores, then a = gamma*rsqrt(var+eps), b = beta - mean*a.  All [128,1].

    fake_cc replaces the collective with a DRAM->DRAM copy so the program
    can run under the single-core TimelineSim (timing studies only)."""
    nc = tc.nc
    small, dram = pools
    alu = mybir.AluOpType
    act = mybir.ActivationFunctionType

    # partial sums per subset: sum = mean*cnt ; sumsq = (var + mean^2)*cnt.
    # Split so the images 0-2 subset aggregates while the conv tail is
    # still running; only the image-3 subset + one add sit on the critical
    # path after the last bn_stats.
    GSPLIT = (NLOC - 1) * NG
    subsets = ((0, GSPLIT), (GSPLIT, NLOC * NG))
    parts = []
    for s, (g0, g1) in enumerate(subsets):
        cnt = float((g1 - g0) * R * W)
        mv = small.tile([C, 2], F32, tag=f"mv{idx}_{s}")
        nc.vector.bn_aggr(mv[:], bnst[:, g0:g1, :])
        e2 = small.tile([C, 1], F32, tag=f"e2{idx}_{s}")
        nc.vector.scalar_tensor_tensor(
            e2[:], mv[:, 0:1], mv[:, 0:1], mv[:, 1:2],
            op0=alu.mult, op1=alu.add)
        p = small.tile([C, 2], F32, tag=f"part{idx}_{s}")
        nc.vector.tensor_scalar_mul(p[:, 0:1], mv[:, 0:1], cnt)
        nc.vector.tensor_scalar_mul(p[:, 1:2], e2[:], cnt)
        parts.append(p)
    part = small.tile([C, 2], F32, tag=f"part{idx}")
    nc.vector.tensor_tensor(part[:], parts[0][:], parts[1][:], op=alu.add)

    cc_in = dram.tile([C, 2], F32, tag=f"ccin{idx}")
    cc_out = dram.tile([C, 2], F32, tag=f"ccout{idx}")
    nc.sync.dma_start(cc_in[:], part[:])
    if fake_cc:
        nc.sync.dma_start(cc_out[:], cc_in[:])
    else:
        nc.gpsimd.collective_compute(
            "AllReduce",
            alu.add,
            replica_groups=[list(range(NCORES))],
            ins=[cc_in[:].opt()],
            outs=[cc_out[:].opt()],
        )
    gl = small.tile([C, 2], F32, tag=f"gl{idx}")
    nc.sync.dma_start(gl[:], cc_out[:])

    # global (sum, sumsq) -> (mean, E[y^2]) in one pass
    gm = small.tile([C, 2], F32, tag=f"gm{idx}")
    nc.vector.tensor_scalar_mul(gm[:], gl[:], 1.0 / float(CNT_GLB))
    mg = gm[:, 0:1]
    # negvar = mean^2 - E[y^2]; std = sqrt(-negvar + eps); inv = 1/std
    negvar = small.tile([C, 1], F32, tag=f"negvar{idx}")
    nc.vector.scalar_tensor_tensor(
        negvar[:], mg, mg, gm[:, 1:2], op0=alu.mult, op1=alu.subtract)
    std = small.tile([C, 1], F32, tag=f"std{idx}")
    nc.scalar.activation(std[:], negvar[:], act.Sqrt,
                         bias=eps_sb[:], scale=-1.0)
    inv = small.tile([C, 1], F32, tag=f"inv{idx}")
    nc.vector.reciprocal(inv[:], std[:])
    a_t = small.tile([C, 1], F32, tag=f"a{idx}")
    nc.vector.tensor_mul(a_t[:], gamma_sb[:], inv[:])
    ma = small.tile([C, 1], F32, tag=f"ma{idx}")
    nc.vector.tensor_mul(ma[:], mg, a_t[:])
    b_t = small.tile([C, 1], F32, tag=f"b{idx}")
    nc.vector.tensor_tensor(b_t[:], beta_sb[:], ma[:], op=alu.subtract)
    return a_t, b_t


def _build_body(tc, xpad_d, w1_d, w2_d, g1_d, b1_d, g2_d, b2_d, out_d,
                reps=1, fake_cc=False):
    nc = tc.nc

    with (
        tc.tile_pool(name="persist", bufs=1) as persist,
        tc.tile_pool(name="small", bufs=1) as small,
        tc.tile_pool(name="dram", bufs=1, space="DRAM") as dram,
        tc.tile_pool(name="fin", bufs=4) as fin,
        tc.tile_pool(name="ostage", bufs=4) as ostage,
        tc.tile_pool(name="psumj", bufs=1, space="PSUM") as psumj,
    ):
        pools = (persist, small, dram, fin, ostage, psumj)
        args = (xpad_d, w1_d, w2_d, g1_d, b1_d, g2_d, b2_d, out_d)
        if reps == 1:
            _emit_iteration(tc, pools, args, fake_cc)
        else:
            with tc.For_i(0, reps, 1):
                _emit_iteration(tc, pools, args, fake_cc)


def _emit_iteration(tc, pools, args, fake_cc):
    nc = tc.nc
    act = mybir.ActivationFunctionType
    alu = mybir.AluOpType
    persist, small, dram, fin, ostage, psumj = pools
    xpad_d, w1_d, w2_d, g1_d, b1_d, g2_d, b2_d, out_d = args
    if True:
        # ---- per-image persistent buffers (x load issued first: the first
        # conv group waits on image 0) ----
        xpad_sb = [persist.tile([C, HP * WP], BF16, tag=f"xp{n}", name=f"xp{n}") for n in range(NLOC)]
        o1p_sb = [persist.tile([C, HP * WP], BF16, tag=f"o1p{n}", name=f"o1p{n}") for n in range(NLOC)]
        y1_sb = [persist.tile([C, H * W], BF16, tag=f"y1_{n}", name=f"y1_{n}") for n in range(NLOC)]
        y2_sb = [persist.tile([C, H * W], BF16, tag=f"y2_{n}", name=f"y2_{n}") for n in range(NLOC)]

        w1_sb = persist.tile([C, 9 * C], BF16, tag="w1")
        w2_sb = persist.tile([C, 9 * C], BF16, tag="w2")
        nc.sync.dma_start(w1_sb[:], w1_d[:])
        # image 0 in three chunks; the first covers rows 0..17 = everything
        # conv1's first batch (groups 0-1) reads
        x0v = xpad_sb[0][:].rearrange("p (h w) -> p h w", w=WP)
        nc.sync.dma_start(x0v[:, 0:18, :], xpad_d[0][:, 0:18, :])
        nc.sync.dma_start(x0v[:, 18:34, :], xpad_d[0][:, 18:34, :])
        nc.sync.dma_start(x0v[:, 34:HP, :], xpad_d[0][:, 34:HP, :])
        nc.sync.dma_start(w2_sb[:], w2_d[:])
        for n in range(1, NLOC):
            nc.sync.dma_start(xpad_sb[n][:], xpad_d[n].rearrange("c h w -> c (h w)"))

        gam1 = persist.tile([C, 1], F32, tag="gam1")
        bet1 = persist.tile([C, 1], F32, tag="bet1")
        gam2 = persist.tile([C, 1], F32, tag="gam2")
        bet2 = persist.tile([C, 1], F32, tag="bet2")
        for t_sb, t_d in ((gam1, g1_d), (bet1, b1_d), (gam2, g2_d), (bet2, b2_d)):
            nc.sync.dma_start(t_sb[:], t_d.rearrange("(c one) -> c one", one=1))
        eps_sb = persist.tile([C, 1], F32, tag="eps")
        nc.vector.memset(eps_sb[:], EPS)

        # ---- ACT table preload (off the critical path): Rsqrt anchors the
        # set used later by the BN coeff chain; Relu/Copy are set fillers.
        tl0 = small.tile([C, 1], F32, tag="tl0")
        nc.scalar.activation(tl0[:], eps_sb[:], act.Sqrt)
        nc.scalar.activation(tl0[:], eps_sb[:], act.Relu)

        # ---- PE warmup: junk matmuls with no dependencies run while the
        # first image chunks stream in, so conv1 starts HAM-warm.
        junk_in = persist.tile([C, FJ], BF16, tag="junk")
        nc.vector.memset(junk_in[:], 0.0)
        junk_ps = [psumj.tile([C, FJ], F32, tag="junkps", name="jp0")]
        _junk_mms(tc, junk_in, junk_ps, WARM0)

        for n in range(NLOC):
            # zero the halo of the conv2 input (interior is written by BN1)
            ov = o1p_sb[n][:].rearrange("p (h w) -> p h w", w=WP)
            nc.vector.memset(ov[:, 0, :], 0.0)
            nc.vector.memset(ov[:, HP - 1, :], 0.0)
            nc.vector.memset(ov[:, 1:HP - 1, 0:XOFF], 0.0)
            nc.vector.memset(ov[:, 1:HP - 1, XOFF + W:WP], 0.0)

        bnst1 = persist.tile([C, NLOC * NG, 6], F32, tag="bnst1")
        bnst2 = persist.tile([C, NLOC * NG, 6], F32, tag="bnst2")

        # ---- conv1 + stats ----
        _conv_phase(tc, w1_sb, xpad_sb, y1_sb, bnst1)
        # keep the PE warm through the sync-BN gap (these have no deps and
        # drain right after conv1's last matmul, well before AR completes)
        _junk_mms(tc, junk_in, junk_ps, WARM1)
        a1, b1 = _bn_coeffs(tc, (small, dram), bnst1, gam1, bet1, eps_sb, 1,
                            fake_cc=fake_cc)

        # ---- out1 = relu(a1*y1 + b1), written into padded conv2 input.
        # Image 0 is split so conv2's first batch (groups 0-1) starts as
        # soon as rows 1..18 are in place. ----
        for n in range(NLOC):
            ov = o1p_sb[n][:].rearrange("p (h w) -> p h w", w=WP)
            yv = y1_sb[n][:].rearrange("p (h w) -> p h w", w=W)
            splits = ((0, 16), (16, H)) if n == 0 else ((0, H),)
            for lo, hi in splits:
                nc.scalar.activation(ov[:, lo + 1:hi + 1, XOFF:XOFF + W],
                                     yv[:, lo:hi, :], act.Relu,
                                     bias=b1[:], scale=a1[:])

        # ---- conv2 + stats ----
        _conv_phase(tc, w2_sb, o1p_sb, y2_sb, bnst2)
        a2, b2 = _bn_coeffs(tc, (small, dram), bnst2, gam2, bet2, eps_sb, 2,
                            fake_cc=fake_cc)

        # ---- out = relu(a2*y2 + b2 + x), bf16, in half-image chunks:
        # u = a2*y2 + b2 runs as tensor_scalar (4x on DVE); the two-tensor
        # +x add and the relu are spread across DVE/Pool/ACT to balance the
        # tail; the store DMA streams out per chunk. ----
        HC = H // 2
        chunks = [(n, h) for n in range(NLOC) for h in range(2)]
        for i, (n, h) in enumerate(chunks):
            xv = xpad_sb[n][:].rearrange("p (h w) -> p h w", w=WP)
            yv = y2_sb[n][:].rearrange("p (h w) -> p h w", w=W)
            r0 = h * HC
            u = fin.tile([C, HC, W], BF16, tag="u")
            nc.vector.tensor_scalar(u[:], yv[:, r0:r0 + HC, :], a2[:], b2[:],
                                    op0=alu.mult, op1=alu.add)
            v = fin.tile([C, HC, W], BF16, tag="v")
            xs = xv[:, r0 + 1:r0 + HC + 1, XOFF:XOFF + W]
            if i % 4 == 3:
                nc.gpsimd.tensor_tensor(v[:], u[:], xs, op=alu.add)
            else:
                nc.vector.tensor_tensor(v[:], u[:], xs, op=alu.add)
            ob = ostage.tile([C, HC, W], BF16, tag="ob")
            if i < 6:
                nc.scalar.activation(ob[:], v[:], act.Relu)
            else:
                nc.vector.tensor_scalar(ob[:], v[:], 1.0, 0.0,
                                        op0=alu.mult, op1=alu.max)
            nc.sync.dma_start(out_d[n, :, r0:r0 + HC, :], ob[:])


def _build_program(reps=1, fake_cc=False, tiny_io=False):
    """tiny_io: timing-only variant -- the heavy tensors become Internal
    DRAM (contents garbage, sizes/DMAs identical) so the benchmark doesn't
    ship ~33MB over the axon tunnel per call.  A tiny real output keeps the
    program well-formed."""
    key = ("nc", reps, fake_cc, tiny_io)
    if key in _CACHE:
        return _CACHE[key]
    nc = bacc.Bacc("TRN2", debug=False, num_devices=NCORES)
    heavy = "Internal" if tiny_io else "ExternalInput"
    heavy_out = "Internal" if tiny_io else "ExternalOutput"
    xpad_d = nc.dram_tensor("xpad", [NLOC, C, HP, WP], BF16, kind=heavy).ap()
    w1_d = nc.dram_tensor("w1t", [C, 9 * C], BF16, kind=heavy).ap()
    w2_d = nc.dram_tensor("w2t", [C, 9 * C], BF16, kind=heavy).ap()
    g1_d = nc.dram_tensor("gamma1", [C], F32, kind="ExternalInput").ap()
    b1_d = nc.dram_tensor("beta1", [C], F32, kind="ExternalInput").ap()
    g2_d = nc.dram_tensor("gamma2", [C], F32, kind="ExternalInput").ap()
    b2_d = nc.dram_tensor("beta2", [C], F32, kind="ExternalInput").ap()
    out_d = nc.dram_tensor("out", [NLOC, C, H, W], BF16, kind=heavy_out).ap()
    if tiny_io:
        tout_d = nc.dram_tensor("tout", [C, 1], F32, kind="ExternalOutput").ap()

    with tile.TileContext(nc) as tc:
        _build_body(tc, xpad_d, w1_d, w2_d, g1_d, b1_d, g2_d, b2_d, out_d,
                    reps=reps, fake_cc=fake_cc)
    nc.compile()
    _CACHE[key] = nc
    return nc


def _prep_in_maps(inputs):
    x = np.asarray(inputs["x"], dtype=np.float32)
    w1 = np.asarray(inputs["w1"], dtype=np.float32)
    w2 = np.asarray(inputs["w2"], dtype=np.float32)

    def wprep(w):
        wb = np.sign(w).astype(np.float32)
        return np.ascontiguousarray(
            wb.transpose(1, 2, 3, 0).reshape(C, 9 * C)).astype(NP_BF16)

    xpad = np.zeros((N, C, HP, WP), dtype=NP_BF16)
    xpad[:, :, 1:H + 1, XOFF:XOFF + W] = x.astype(NP_BF16)

    common = {
        "w1t": wprep(w1),
        "w2t": wprep(w2),
        "gamma1": np.asarray(inputs["gamma1"], np.float32),
        "beta1": np.asarray(inputs["beta1"], np.float32),
        "gamma2": np.asarray(inputs["gamma2"], np.float32),
        "beta2": np.asarray(inputs["beta2"], np.float32),
    }
    return [
        {"xpad": np.ascontiguousarray(xpad[k * NLOC:(k + 1) * NLOC]), **common}
        for k in range(NCORES)
    ]


def _run(inputs, trace=False, trace_kwargs=None, reps=1):
    in_maps = _prep_in_maps(inputs)
    nc = _build_program(reps=reps)
    res = run_bass_kernel_spmd(
        nc, in_maps, core_ids=list(range(NCORES)), trace=trace,
        **(trace_kwargs or {}))
    out = np.concatenate(
        [res.results[k]["out"].astype(np.float32) for k in range(NCORES)],
        axis=0)
    return out, res


def kernel(**inputs) -> np.ndarray:
    out, _ = _run(inputs, trace=False)
    return out


# revision 34
# speedup vs baseline: 14.0624x; 1.0389x over previous
"""Trainium2 Bass kernel for a binarized-conv ResNet BasicBlock.

    y1 = conv3x3(x, sign(w1)); out1 = relu(BN(y1))
    y2 = conv3x3(out1, sign(w2)); out = relu(BN(y2) + x)

BN is training-mode (batch stats over N,H,W). Sharding: data-parallel over
the batch (N=32 -> 4 images per core on 8 cores); conv weights + BN params
replicated; BN stats sync'd with a tiny [128,2] collective (sum, sumsq).

Conv mapping: C_in=128 lands exactly on the 128 SBUF partitions; a 3x3
conv is 9 shifted matmuls accumulated in PSUM (lhsT = [Cin, Cout] per tap,
rhs = padded input rows). Matmul inputs are bf16 (weights are exactly
+/-1); accumulation is fp32 in PSUM.  Padded width is 60 with the interior
at columns 2..57 so every row of the interior is 4-byte aligned (keeps the
DVE/ACT 2x packed modes on the elementwise passes).  The kernel output is
bf16 (upcast to fp32 on the host) to halve the store traffic.
"""

import numpy as np

import concourse.bass as bass
import concourse.tile as tile
from concourse import bacc, mybir
from concourse.bass_utils import run_bass_kernel_spmd

F32 = mybir.dt.float32
BF16 = mybir.dt.bfloat16
NP_BF16 = mybir.dt.np(BF16)

N, C, H, W = 32, 128, 56, 56
NCORES = 8
NLOC = N // NCORES           # images per core
HP, WP = H + 2, H + 4        # padded spatial dims; interior at [1:57, 2:58]
XOFF = 2                     # interior column offset (4B alignment)
R = 8                        # output rows per matmul group
NG = H // R                  # groups per image
F = R * W                    # moving free dim per matmul (448 <= 512)
CNT_GLB = N * H * W          # global BN count
EPS = 1e-5
WARM0 = 16                   # warmup junk matmuls before conv1
WARM1 = 52                   # keep-warm junk matmuls during the BN1 sync

_CACHE = {}


def _conv_phase(tc, w_sb, src_pads, dst_ys, bnst, gb=2):
    """One conv layer: 9-tap matmul accumulation per (image, row-group),
    PSUM evicted to SBUF bf16 via ACT copy, then bn_stats on the evicted
    tile for the sync-BN statistics."""
    nc = tc.nc
    groups = [(n, g) for n in range(NLOC) for g in range(NG)]
    xvs = [src_pads[n][:].rearrange("p (h w) -> p h w", w=WP) for n in range(NLOC)]
    with tc.tile_pool(name="psum", bufs=6, space="PSUM") as psum:
        for b0 in range(0, len(groups), gb):
            batch = groups[b0:b0 + gb]
            tiles = [psum.tile([C, F], F32, tag="ps", name=f"ps{b0 + i}")
                     for i in range(len(batch))]
            t = 0
            for ky in range(3):
                for kx in range(3):
                    for i, (n, g) in enumerate(batch):
                        r0 = g * R
                        nc.tensor.matmul(
                            tiles[i][:],
                            w_sb[:, t * C:(t + 1) * C],
                            xvs[n][:, r0 + ky:r0 + ky + R, kx + 1:kx + 1 + W],
                            start=(t == 0),
                            stop=(t == 8),
                        )
                    t += 1
            for i, (n, g) in enumerate(batch):
                r0 = g * R
                seg = dst_ys[n][:][:, r0 * W:(r0 + R) * W]
                nc.scalar.copy(seg, tiles[i][:])
                nc.vector.bn_stats(bnst[:, n * NG + g, :], seg)


FJ = 448                     # junk matmul free dim


def _junk_mms(tc, junk_in, junk_ps, count):
    """Dependency-free matmuls that keep the PE HAM-warm while it would
    otherwise idle (initial DMA wait, sync-BN collective)."""
    nc = tc.nc
    for k in range(count):
        nc.tensor.matmul(junk_ps[0][:], junk_in[:, 0:C],
                         junk_in[:, 0:FJ],
                         start=True, stop=True, skip_group_check=True)


def _bn_coeffs(tc, pools, bnst, gamma_sb, beta_sb, eps_sb, idx, fake_cc=False):
    """Local (mean,var) -> (sum,sumsq) partials, AllReduce across the 8
    cores, then a = gamma*rsqrt(var+eps), b = beta - mean*a.  All [128,1].

    fake_cc replaces the collective with a DRAM->DRAM copy so the program
    can run under the single-core TimelineSim (timing studies only)."""
    nc = tc.nc
    small, dram = pools
    alu = mybir.AluOpType
    act = mybir.ActivationFunctionType

    # partial sums per subset: sum = mean*cnt ; sumsq = (var + mean^2)*cnt.
    # Split so the images 0-2 subset aggregates while the conv tail is
    # still running; only the image-3 subset + one add sit on the critical
    # path after the last bn_stats.
    GSPLIT = (NLOC - 1) * NG
    subsets = ((0, GSPLIT), (GSPLIT, NLOC * NG))
    parts = []
    for s, (g0, g1) in enumerate(subsets):
        cnt = float((g1 - g0) * R * W)
        mv = small.tile([C, 2], F32, tag=f"mv{idx}_{s}")
        nc.vector.bn_aggr(mv[:], bnst[:, g0:g1, :])
        e2 = small.tile([C, 1], F32, tag=f"e2{idx}_{s}")
        nc.vector.scalar_tensor_tensor(
            e2[:], mv[:, 0:1], mv[:, 0:1], mv[:, 1:2],
            op0=alu.mult, op1=alu.add)
        p = small.tile([C, 2], F32, tag=f"part{idx}_{s}")
        nc.vector.tensor_scalar_mul(p[:, 0:1], mv[:, 0:1], cnt)
        nc.vector.tensor_scalar_mul(p[:, 1:2], e2[:], cnt)
        parts.append(p)
    part = small.tile([C, 2], F32, tag=f"part{idx}")
    nc.vector.tensor_tensor(part[:], parts[0][:], parts[1][:], op=alu.add)

    cc_in = dram.tile([C, 2], F32, tag=f"ccin{idx}")
    cc_out = dram.tile([C, 2], F32, tag=f"ccout{idx}")
    nc.sync.dma_start(cc_in[:], part[:])
    if fake_cc:
        nc.sync.dma_start(cc_out[:], cc_in[:])
    else:
        nc.gpsimd.collective_compute(
            "AllReduce",
            alu.add,
            replica_groups=[list(range(NCORES))],
            ins=[cc_in[:].opt()],
            outs=[cc_out[:].opt()],
        )
    gl = small.tile([C, 2], F32, tag=f"gl{idx}")
    nc.sync.dma_start(gl[:], cc_out[:])

    # global (sum, sumsq) -> (mean, E[y^2]) in one pass
    gm = small.tile([C, 2], F32, tag=f"gm{idx}")
    nc.vector.tensor_scalar_mul(gm[:], gl[:], 1.0 / float(CNT_GLB))
    mg = gm[:, 0:1]
    # negvar = mean^2 - E[y^2]; std = sqrt(-negvar + eps); inv = 1/std
    negvar = small.tile([C, 1], F32, tag=f"negvar{idx}")
    nc.vector.scalar_tensor_tensor(
        negvar[:], mg, mg, gm[:, 1:2], op0=alu.mult, op1=alu.subtract)
    std = small.tile([C, 1], F32, tag=f"std{idx}")
    nc.scalar.activation(std[:], negvar[:], act.Sqrt,
                         bias=eps_sb[:], scale=-1.0)
    inv = small.tile([C, 1], F32, tag=f"inv{idx}")
    nc.vector.reciprocal(inv[:], std[:])
    a_t = small.tile([C, 1], F32, tag=f"a{idx}")
    nc.vector.tensor_mul(a_t[:], gamma_sb[:], inv[:])
    ma = small.tile([C, 1], F32, tag=f"ma{idx}")
    nc.vector.tensor_mul(ma[:], mg, a_t[:])
    b_t = small.tile([C, 1], F32, tag=f"b{idx}")
    nc.vector.tensor_tensor(b_t[:], beta_sb[:], ma[:], op=alu.subtract)
    return a_t, b_t


def _build_body(tc, xpad_d, w1_d, w2_d, g1_d, b1_d, g2_d, b2_d, out_d,
                reps=1, fake_cc=False):
    nc = tc.nc

    with (
        tc.tile_pool(name="persist", bufs=1) as persist,
        tc.tile_pool(name="small", bufs=1) as small,
        tc.tile_pool(name="dram", bufs=1, space="DRAM") as dram,
        tc.tile_pool(name="fin", bufs=4) as fin,
        tc.tile_pool(name="ostage", bufs=4) as ostage,
        tc.tile_pool(name="psumj", bufs=1, space="PSUM") as psumj,
    ):
        pools = (persist, small, dram, fin, ostage, psumj)
        args = (xpad_d, w1_d, w2_d, g1_d, b1_d, g2_d, b2_d, out_d)
        if reps == 1:
            _emit_iteration(tc, pools, args, fake_cc)
        else:
            with tc.For_i(0, reps, 1):
                _emit_iteration(tc, pools, args, fake_cc)


def _emit_iteration(tc, pools, args, fake_cc):
    nc = tc.nc
    act = mybir.ActivationFunctionType
    alu = mybir.AluOpType
    persist, small, dram, fin, ostage, psumj = pools
    xpad_d, w1_d, w2_d, g1_d, b1_d, g2_d, b2_d, out_d = args
    if True:
        # ---- per-image persistent buffers (x load issued first: the first
        # conv group waits on image 0) ----
        xpad_sb = [persist.tile([C, HP * WP], BF16, tag=f"xp{n}", name=f"xp{n}") for n in range(NLOC)]
        o1p_sb = [persist.tile([C, HP * WP], BF16, tag=f"o1p{n}", name=f"o1p{n}") for n in range(NLOC)]
        y1_sb = [persist.tile([C, H * W], BF16, tag=f"y1_{n}", name=f"y1_{n}") for n in range(NLOC)]
        y2_sb = [persist.tile([C, H * W], BF16, tag=f"y2_{n}", name=f"y2_{n}") for n in range(NLOC)]

        w1_sb = persist.tile([C, 9 * C], BF16, tag="w1")
        w2_sb = persist.tile([C, 9 * C], BF16, tag="w2")
        nc.sync.dma_start(w1_sb[:], w1_d[:])
        # image 0 in three chunks; the first covers rows 0..17 = everything
        # conv1's first batch (groups 0-1) reads
        x0v = xpad_sb[0][:].rearrange("p (h w) -> p h w", w=WP)
        nc.sync.dma_start(x0v[:, 0:18, :], xpad_d[0][:, 0:18, :])
        nc.sync.dma_start(x0v[:, 18:34, :], xpad_d[0][:, 18:34, :])
        nc.sync.dma_start(x0v[:, 34:HP, :], xpad_d[0][:, 34:HP, :])
        nc.sync.dma_start(w2_sb[:], w2_d[:])
        for n in range(1, NLOC):
            nc.sync.dma_start(xpad_sb[n][:], xpad_d[n].rearrange("c h w -> c (h w)"))

        gam1 = persist.tile([C, 1], F32, tag="gam1")
        bet1 = persist.tile([C, 1], F32, tag="bet1")
        gam2 = persist.tile([C, 1], F32, tag="gam2")
        bet2 = persist.tile([C, 1], F32, tag="bet2")
        for t_sb, t_d in ((gam1, g1_d), (bet1, b1_d), (gam2, g2_d), (bet2, b2_d)):
            nc.sync.dma_start(t_sb[:], t_d.rearrange("(c one) -> c one", one=1))
        eps_sb = persist.tile([C, 1], F32, tag="eps")
        nc.vector.memset(eps_sb[:], EPS)

        # ---- ACT table preload (off the critical path): Rsqrt anchors the
        # set used later by the BN coeff chain; Relu/Copy are set fillers.
        tl0 = small.tile([C, 1], F32, tag="tl0")
        nc.scalar.activation(tl0[:], eps_sb[:], act.Sqrt)
        nc.scalar.activation(tl0[:], eps_sb[:], act.Relu)

        # ---- PE warmup: junk matmuls with no dependencies run while the
        # first image chunks stream in, so conv1 starts HAM-warm.
        junk_in = persist.tile([C, FJ], BF16, tag="junk")
        nc.vector.memset(junk_in[:], 0.0)
        junk_ps = [psumj.tile([C, FJ], F32, tag="junkps", name="jp0")]
        _junk_mms(tc, junk_in, junk_ps, WARM0)

        for n in range(NLOC):
            # zero the halo of the conv2 input (interior is written by BN1)
            ov = o1p_sb[n][:].rearrange("p (h w) -> p h w", w=WP)
            nc.vector.memset(ov[:, 0, :], 0.0)
            nc.vector.memset(ov[:, HP - 1, :], 0.0)
            nc.vector.memset(ov[:, 1:HP - 1, 0:XOFF], 0.0)
            nc.vector.memset(ov[:, 1:HP - 1, XOFF + W:WP], 0.0)

        bnst1 = persist.tile([C, NLOC * NG, 6], F32, tag="bnst1")
        bnst2 = persist.tile([C, NLOC * NG, 6], F32, tag="bnst2")

        # ---- conv1 + stats ----
        _conv_phase(tc, w1_sb, xpad_sb, y1_sb, bnst1)
        # keep the PE warm through the sync-BN gap (these have no deps and
        # drain right after conv1's last matmul, well before AR completes)
        _junk_mms(tc, junk_in, junk_ps, WARM1)
        a1, b1 = _bn_coeffs(tc, (small, dram), bnst1, gam1, bet1, eps_sb, 1,
                            fake_cc=fake_cc)

        # ---- out1 = relu(a1*y1 + b1), written into padded conv2 input.
        # Image 0 is split so conv2's first batch (groups 0-1) starts as
        # soon as rows 1..18 are in place. ----
        for n in range(NLOC):
            ov = o1p_sb[n][:].rearrange("p (h w) -> p h w", w=WP)
            yv = y1_sb[n][:].rearrange("p (h w) -> p h w", w=W)
            splits = ((0, 16), (16, H)) if n == 0 else ((0, H),)
            for lo, hi in splits:
                nc.scalar.activation(ov[:, lo + 1:hi + 1, XOFF:XOFF + W],
                                     yv[:, lo:hi, :], act.Relu,
                                     bias=b1[:], scale=a1[:])

        # ---- conv2 + stats ----
        _conv_phase(tc, w2_sb, o1p_sb, y2_sb, bnst2)
        a2, b2 = _bn_coeffs(tc, (small, dram), bnst2, gam2, bet2, eps_sb, 2,
                            fake_cc=fake_cc)

        # ---- out = relu(a2*y2 + b2 + x), bf16, in half-image chunks:
        # u = a2*y2 + b2 runs as tensor_scalar (4x on DVE); the two-tensor
        # +x add and the relu are spread across DVE/Pool/ACT to balance the
        # tail; the store DMA streams out per chunk. ----
        HC = H // 2
        chunks = [(n, h) for n in range(NLOC) for h in range(2)]
        for i, (n, h) in enumerate(chunks):
            xv = xpad_sb[n][:].rearrange("p (h w) -> p h w", w=WP)
            yv = y2_sb[n][:].rearrange("p (h w) -> p h w", w=W)
            r0 = h * HC
            u = fin.tile([C, HC, W], BF16, tag="u")
            nc.vector.tensor_scalar(u[:], yv[:, r0:r0 + HC, :], a2[:], b2[:],
                                    op0=alu.mult, op1=alu.add)
            v = fin.tile([C, HC, W], BF16, tag="v")
            xs = xv[:, r0 + 1:r0 + HC + 1, XOFF:XOFF + W]
            if i % 4 == 3:
                nc.gpsimd.tensor_tensor(v[:], u[:], xs, op=alu.add)
            else:
                nc.vector.tensor_tensor(v[:], u[:], xs, op=alu.add)
            ob = ostage.tile([C, HC, W], BF16, tag="ob")
            if i < 6:
                nc.scalar.activation(ob[:], v[:], act.Relu)
            else:
                nc.vector.tensor_scalar(ob[:], v[:], 1.0, 0.0,
                                        op0=alu.mult, op1=alu.max)
            nc.sync.dma_start(out_d[n, :, r0:r0 + HC, :], ob[:])


def _build_program(reps=1, fake_cc=False, tiny_io=False):
    """tiny_io: timing-only variant -- the heavy tensors become Internal
    DRAM (contents garbage, sizes/DMAs identical) so the benchmark doesn't
    ship ~33MB over the axon tunnel per call.  A tiny real output keeps the
    program well-formed."""
    key = ("nc", reps, fake_cc, tiny_io)
    if key in _CACHE:
        return _CACHE[key]
    nc = bacc.Bacc("TRN2", debug=False, num_devices=NCORES)
    heavy = "Internal" if tiny_io else "ExternalInput"
    heavy_out = "Internal" if tiny_io else "ExternalOutput"
    xpad_d = nc.dram_tensor("xpad", [NLOC, C, HP, WP], BF16, kind=heavy).ap()
    w1_d = nc.dram_tensor("w1t", [C, 9 * C], BF16, kind=heavy).ap()
    w2_d = nc.dram_tensor("w2t", [C, 9 * C], BF16, kind=heavy).ap()
    g1_d = nc.dram_tensor("gamma1", [C], F32, kind="ExternalInput").ap()
    b1_d = nc.dram_tensor("beta1", [C], F32, kind="ExternalInput").ap()
    g2_d = nc.dram_tensor("gamma2", [C], F32, kind="ExternalInput").ap()
    b2_d = nc.dram_tensor("beta2", [C], F32, kind="ExternalInput").ap()
    out_d = nc.dram_tensor("out", [NLOC, C, H, W], BF16, kind=heavy_out).ap()
    if tiny_io:
        tout_d = nc.dram_tensor("tout", [C, 1], F32, kind="ExternalOutput").ap()

    with tile.TileContext(nc) as tc:
        _build_body(tc, xpad_d, w1_d, w2_d, g1_d, b1_d, g2_d, b2_d, out_d,
                    reps=reps, fake_cc=fake_cc)
    nc.compile()
    _CACHE[key] = nc
    return nc


def _prep_in_maps(inputs):
    x = np.asarray(inputs["x"], dtype=np.float32)
    w1 = np.asarray(inputs["w1"], dtype=np.float32)
    w2 = np.asarray(inputs["w2"], dtype=np.float32)

    def wprep(w):
        wb = np.sign(w).astype(np.float32)
        return np.ascontiguousarray(
            wb.transpose(1, 2, 3, 0).reshape(C, 9 * C)).astype(NP_BF16)

    xpad = np.zeros((N, C, HP, WP), dtype=NP_BF16)
    xpad[:, :, 1:H + 1, XOFF:XOFF + W] = x.astype(NP_BF16)

    common = {
        "w1t": wprep(w1),
        "w2t": wprep(w2),
        "gamma1": np.asarray(inputs["gamma1"], np.float32),
        "beta1": np.asarray(inputs["beta1"], np.float32),
        "gamma2": np.asarray(inputs["gamma2"], np.float32),
        "beta2": np.asarray(inputs["beta2"], np.float32),
    }
    return [
        {"xpad": np.ascontiguousarray(xpad[k * NLOC:(k + 1) * NLOC]), **common}
        for k in range(NCORES)
    ]


def _run(inputs, trace=False, trace_kwargs=None, reps=1):
    in_maps = _prep_in_maps(inputs)
    nc = _build_program(reps=reps)
    res = run_bass_kernel_spmd(
        nc, in_maps, core_ids=list(range(NCORES)), trace=trace,
        **(trace_kwargs or {}))
    out = np.concatenate(
        [res.results[k]["out"].astype(np.float32) for k in range(NCORES)],
        axis=0)
    return out, res


def kernel(**inputs) -> np.ndarray:
    out, _ = _run(inputs, trace=False)
    return out


# revision 35
# speedup vs baseline: 14.2464x; 1.0131x over previous
"""Trainium2 Bass kernel for a binarized-conv ResNet BasicBlock.

    y1 = conv3x3(x, sign(w1)); out1 = relu(BN(y1))
    y2 = conv3x3(out1, sign(w2)); out = relu(BN(y2) + x)

BN is training-mode (batch stats over N,H,W). Sharding: data-parallel over
the batch (N=32 -> 4 images per core on 8 cores); conv weights + BN params
replicated; BN stats sync'd with a tiny [128,2] collective (sum, sumsq).

Conv mapping: C_in=128 lands exactly on the 128 SBUF partitions; a 3x3
conv is 9 shifted matmuls accumulated in PSUM (lhsT = [Cin, Cout] per tap,
rhs = padded input rows). Matmul inputs are bf16 (weights are exactly
+/-1); accumulation is fp32 in PSUM.  Padded width is 60 with the interior
at columns 2..57 so every row of the interior is 4-byte aligned (keeps the
DVE/ACT 2x packed modes on the elementwise passes).  The kernel output is
bf16 (upcast to fp32 on the host) to halve the store traffic.
"""

import numpy as np

import concourse.bass as bass
import concourse.tile as tile
from concourse import bacc, mybir
from concourse.bass_utils import run_bass_kernel_spmd

F32 = mybir.dt.float32
BF16 = mybir.dt.bfloat16
NP_BF16 = mybir.dt.np(BF16)

N, C, H, W = 32, 128, 56, 56
NCORES = 8
NLOC = N // NCORES           # images per core
HP, WP = H + 2, H + 4        # padded spatial dims; interior at [1:57, 2:58]
XOFF = 2                     # interior column offset (4B alignment)
R = 8                        # output rows per matmul group
NG = H // R                  # groups per image
F = R * W                    # moving free dim per matmul (448 <= 512)
CNT_GLB = N * H * W          # global BN count
EPS = 1e-5
WARM0 = 16                   # warmup junk matmuls before conv1
WARM1 = 44                   # keep-warm junk matmuls during the BN1 sync

_CACHE = {}


def _conv_phase(tc, w_sb, src_pads, dst_ys, bnst, gb=2):
    """One conv layer: 9-tap matmul accumulation per (image, row-group),
    PSUM evicted to SBUF bf16 via ACT copy, then bn_stats on the evicted
    tile for the sync-BN statistics."""
    nc = tc.nc
    groups = [(n, g) for n in range(NLOC) for g in range(NG)]
    xvs = [src_pads[n][:].rearrange("p (h w) -> p h w", w=WP) for n in range(NLOC)]
    with tc.tile_pool(name="psum", bufs=6, space="PSUM") as psum:
        for b0 in range(0, len(groups), gb):
            batch = groups[b0:b0 + gb]
            tiles = [psum.tile([C, F], F32, tag="ps", name=f"ps{b0 + i}")
                     for i in range(len(batch))]
            t = 0
            for ky in range(3):
                for kx in range(3):
                    for i, (n, g) in enumerate(batch):
                        r0 = g * R
                        nc.tensor.matmul(
                            tiles[i][:],
                            w_sb[:, t * C:(t + 1) * C],
                            xvs[n][:, r0 + ky:r0 + ky + R, kx + 1:kx + 1 + W],
                            start=(t == 0),
                            stop=(t == 8),
                        )
                    t += 1
            for i, (n, g) in enumerate(batch):
                r0 = g * R
                seg = dst_ys[n][:][:, r0 * W:(r0 + R) * W]
                nc.scalar.copy(seg, tiles[i][:])
                nc.vector.bn_stats(bnst[:, n * NG + g, :], seg)


FJ = 448                     # junk matmul free dim


def _junk_mms(tc, junk_in, junk_ps, count):
    """Dependency-free matmuls that keep the PE HAM-warm while it would
    otherwise idle (initial DMA wait, sync-BN collective)."""
    nc = tc.nc
    for k in range(count):
        nc.tensor.matmul(junk_ps[0][:], junk_in[:, 0:C],
                         junk_in[:, 0:FJ],
                         start=True, stop=True, skip_group_check=True)


def _bn_coeffs(tc, pools, bnst, gamma_sb, beta_sb, eps_sb, idx, fake_cc=False):
    """Local (mean,var) -> (sum,sumsq) partials, AllReduce across the 8
    cores, then a = gamma*rsqrt(var+eps), b = beta - mean*a.  All [128,1].

    fake_cc replaces the collective with a DRAM->DRAM copy so the program
    can run under the single-core TimelineSim (timing studies only)."""
    nc = tc.nc
    small, dram = pools
    alu = mybir.AluOpType
    act = mybir.ActivationFunctionType

    # partial sums per subset: sum = mean*cnt ; sumsq = (var + mean^2)*cnt.
    # Split so the images 0-2 subset aggregates while the conv tail is
    # still running; only the image-3 subset + one add sit on the critical
    # path after the last bn_stats.
    GSPLIT = (NLOC - 1) * NG
    subsets = ((0, GSPLIT), (GSPLIT, NLOC * NG))
    parts = []
    for s, (g0, g1) in enumerate(subsets):
        cnt = float((g1 - g0) * R * W)
        mv = small.tile([C, 2], F32, tag=f"mv{idx}_{s}")
        nc.vector.bn_aggr(mv[:], bnst[:, g0:g1, :])
        e2 = small.tile([C, 1], F32, tag=f"e2{idx}_{s}")
        nc.vector.scalar_tensor_tensor(
            e2[:], mv[:, 0:1], mv[:, 0:1], mv[:, 1:2],
            op0=alu.mult, op1=alu.add)
        p = small.tile([C, 2], F32, tag=f"part{idx}_{s}")
        nc.vector.tensor_scalar_mul(p[:, 0:1], mv[:, 0:1], cnt)
        nc.vector.tensor_scalar_mul(p[:, 1:2], e2[:], cnt)
        parts.append(p)
    part = small.tile([C, 2], F32, tag=f"part{idx}")
    nc.vector.tensor_tensor(part[:], parts[0][:], parts[1][:], op=alu.add)

    cc_in = dram.tile([C, 2], F32, tag=f"ccin{idx}")
    cc_out = dram.tile([C, 2], F32, tag=f"ccout{idx}")
    nc.sync.dma_start(cc_in[:], part[:])
    if fake_cc:
        nc.sync.dma_start(cc_out[:], cc_in[:])
    else:
        nc.gpsimd.collective_compute(
            "AllReduce",
            alu.add,
            replica_groups=[list(range(NCORES))],
            ins=[cc_in[:].opt()],
            outs=[cc_out[:].opt()],
        )
    gl = small.tile([C, 2], F32, tag=f"gl{idx}")
    nc.sync.dma_start(gl[:], cc_out[:])

    # global (sum, sumsq) -> (mean, E[y^2]) in one pass
    gm = small.tile([C, 2], F32, tag=f"gm{idx}")
    nc.vector.tensor_scalar_mul(gm[:], gl[:], 1.0 / float(CNT_GLB))
    mg = gm[:, 0:1]
    # negvar = mean^2 - E[y^2]; std = sqrt(-negvar + eps); inv = 1/std
    negvar = small.tile([C, 1], F32, tag=f"negvar{idx}")
    nc.vector.scalar_tensor_tensor(
        negvar[:], mg, mg, gm[:, 1:2], op0=alu.mult, op1=alu.subtract)
    std = small.tile([C, 1], F32, tag=f"std{idx}")
    nc.scalar.activation(std[:], negvar[:], act.Sqrt,
                         bias=eps_sb[:], scale=-1.0)
    inv = small.tile([C, 1], F32, tag=f"inv{idx}")
    nc.vector.reciprocal(inv[:], std[:])
    a_t = small.tile([C, 1], F32, tag=f"a{idx}")
    nc.vector.tensor_mul(a_t[:], gamma_sb[:], inv[:])
    ma = small.tile([C, 1], F32, tag=f"ma{idx}")
    nc.vector.tensor_mul(ma[:], mg, a_t[:])
    b_t = small.tile([C, 1], F32, tag=f"b{idx}")
    nc.vector.tensor_tensor(b_t[:], beta_sb[:], ma[:], op=alu.subtract)
    return a_t, b_t


def _build_body(tc, xpad_d, w1_d, w2_d, g1_d, b1_d, g2_d, b2_d, out_d,
                reps=1, fake_cc=False):
    nc = tc.nc

    with (
        tc.tile_pool(name="persist", bufs=1) as persist,
        tc.tile_pool(name="small", bufs=1) as small,
        tc.tile_pool(name="dram", bufs=1, space="DRAM") as dram,
        tc.tile_pool(name="fin", bufs=4) as fin,
        tc.tile_pool(name="ostage", bufs=4) as ostage,
        tc.tile_pool(name="psumj", bufs=1, space="PSUM") as psumj,
    ):
        pools = (persist, small, dram, fin, ostage, psumj)
        args = (xpad_d, w1_d, w2_d, g1_d, b1_d, g2_d, b2_d, out_d)
        if reps == 1:
            _emit_iteration(tc, pools, args, fake_cc)
        else:
            with tc.For_i(0, reps, 1):
                _emit_iteration(tc, pools, args, fake_cc)


def _emit_iteration(tc, pools, args, fake_cc):
    nc = tc.nc
    act = mybir.ActivationFunctionType
    alu = mybir.AluOpType
    persist, small, dram, fin, ostage, psumj = pools
    xpad_d, w1_d, w2_d, g1_d, b1_d, g2_d, b2_d, out_d = args
    if True:
        # ---- per-image persistent buffers (x load issued first: the first
        # conv group waits on image 0) ----
        xpad_sb = [persist.tile([C, HP * WP], BF16, tag=f"xp{n}", name=f"xp{n}") for n in range(NLOC)]
        o1p_sb = [persist.tile([C, HP * WP], BF16, tag=f"o1p{n}", name=f"o1p{n}") for n in range(NLOC)]
        y1_sb = [persist.tile([C, H * W], BF16, tag=f"y1_{n}", name=f"y1_{n}") for n in range(NLOC)]
        y2_sb = [persist.tile([C, H * W], BF16, tag=f"y2_{n}", name=f"y2_{n}") for n in range(NLOC)]

        w1_sb = persist.tile([C, 9 * C], BF16, tag="w1")
        w2_sb = persist.tile([C, 9 * C], BF16, tag="w2")
        nc.sync.dma_start(w1_sb[:], w1_d[:])
        # image 0 in three chunks; the first covers rows 0..17 = everything
        # conv1's first batch (groups 0-1) reads
        x0v = xpad_sb[0][:].rearrange("p (h w) -> p h w", w=WP)
        nc.sync.dma_start(x0v[:, 0:18, :], xpad_d[0][:, 0:18, :])
        nc.sync.dma_start(x0v[:, 18:34, :], xpad_d[0][:, 18:34, :])
        nc.sync.dma_start(x0v[:, 34:HP, :], xpad_d[0][:, 34:HP, :])
        nc.sync.dma_start(w2_sb[:], w2_d[:])
        for n in range(1, NLOC):
            nc.sync.dma_start(xpad_sb[n][:], xpad_d[n].rearrange("c h w -> c (h w)"))

        gam1 = persist.tile([C, 1], F32, tag="gam1")
        bet1 = persist.tile([C, 1], F32, tag="bet1")
        gam2 = persist.tile([C, 1], F32, tag="gam2")
        bet2 = persist.tile([C, 1], F32, tag="bet2")
        for t_sb, t_d in ((gam1, g1_d), (bet1, b1_d), (gam2, g2_d), (bet2, b2_d)):
            nc.sync.dma_start(t_sb[:], t_d.rearrange("(c one) -> c one", one=1))
        eps_sb = persist.tile([C, 1], F32, tag="eps")
        nc.vector.memset(eps_sb[:], EPS)

        # ---- ACT table preload (off the critical path): Rsqrt anchors the
        # set used later by the BN coeff chain; Relu/Copy are set fillers.
        tl0 = small.tile([C, 1], F32, tag="tl0")
        nc.scalar.activation(tl0[:], eps_sb[:], act.Sqrt)
        nc.scalar.activation(tl0[:], eps_sb[:], act.Relu)

        # ---- PE warmup: junk matmuls with no dependencies run while the
        # first image chunks stream in, so conv1 starts HAM-warm.
        junk_in = persist.tile([C, FJ], BF16, tag="junk")
        nc.vector.memset(junk_in[:], 0.0)
        junk_ps = [psumj.tile([C, FJ], F32, tag="junkps", name="jp0")]
        _junk_mms(tc, junk_in, junk_ps, WARM0)

        for n in range(NLOC):
            # zero the halo of the conv2 input (interior is written by BN1)
            ov = o1p_sb[n][:].rearrange("p (h w) -> p h w", w=WP)
            nc.vector.memset(ov[:, 0, :], 0.0)
            nc.vector.memset(ov[:, HP - 1, :], 0.0)
            nc.vector.memset(ov[:, 1:HP - 1, 0:XOFF], 0.0)
            nc.vector.memset(ov[:, 1:HP - 1, XOFF + W:WP], 0.0)

        bnst1 = persist.tile([C, NLOC * NG, 6], F32, tag="bnst1")
        bnst2 = persist.tile([C, NLOC * NG, 6], F32, tag="bnst2")

        # ---- conv1 + stats ----
        _conv_phase(tc, w1_sb, xpad_sb, y1_sb, bnst1)
        # keep the PE warm through the sync-BN gap (these have no deps and
        # drain right after conv1's last matmul, well before AR completes)
        _junk_mms(tc, junk_in, junk_ps, WARM1)
        a1, b1 = _bn_coeffs(tc, (small, dram), bnst1, gam1, bet1, eps_sb, 1,
                            fake_cc=fake_cc)

        # ---- out1 = relu(a1*y1 + b1), written into padded conv2 input.
        # Image 0 is split so conv2's first batch (groups 0-1) starts as
        # soon as rows 1..18 are in place. ----
        for n in range(NLOC):
            ov = o1p_sb[n][:].rearrange("p (h w) -> p h w", w=WP)
            yv = y1_sb[n][:].rearrange("p (h w) -> p h w", w=W)
            splits = ((0, 16), (16, H)) if n == 0 else ((0, H),)
            for lo, hi in splits:
                nc.scalar.activation(ov[:, lo + 1:hi + 1, XOFF:XOFF + W],
                                     yv[:, lo:hi, :], act.Relu,
                                     bias=b1[:], scale=a1[:])

        # ---- conv2 + stats ----
        _conv_phase(tc, w2_sb, o1p_sb, y2_sb, bnst2)
        a2, b2 = _bn_coeffs(tc, (small, dram), bnst2, gam2, bet2, eps_sb, 2,
                            fake_cc=fake_cc)

        # ---- out = relu(a2*y2 + b2 + x), bf16, in half-image chunks:
        # u = a2*y2 + b2 runs as tensor_scalar (4x on DVE); the two-tensor
        # +x add and the relu are spread across DVE/Pool/ACT to balance the
        # tail; the store DMA streams out per chunk. ----
        HC = H // 2
        chunks = [(n, h) for n in range(NLOC) for h in range(2)]
        for i, (n, h) in enumerate(chunks):
            xv = xpad_sb[n][:].rearrange("p (h w) -> p h w", w=WP)
            yv = y2_sb[n][:].rearrange("p (h w) -> p h w", w=W)
            r0 = h * HC
            u = fin.tile([C, HC, W], BF16, tag="u")
            nc.vector.tensor_scalar(u[:], yv[:, r0:r0 + HC, :], a2[:], b2[:],
                                    op0=alu.mult, op1=alu.add)
            v = fin.tile([C, HC, W], BF16, tag="v")
            xs = xv[:, r0 + 1:r0 + HC + 1, XOFF:XOFF + W]
            if i % 4 == 3:
                nc.gpsimd.tensor_tensor(v[:], u[:], xs, op=alu.add)
            else:
                nc.vector.tensor_tensor(v[:], u[:], xs, op=alu.add)
            ob = ostage.tile([C, HC, W], BF16, tag="ob")
            if i < 6:
                nc.scalar.activation(ob[:], v[:], act.Relu)
            else:
                nc.vector.tensor_scalar(ob[:], v[:], 1.0, 0.0,
                                        op0=alu.mult, op1=alu.max)
            nc.sync.dma_start(out_d[n, :, r0:r0 + HC, :], ob[:])


def _build_program(reps=1, fake_cc=False, tiny_io=False):
    """tiny_io: timing-only variant -- the heavy tensors become Internal
    DRAM (contents garbage, sizes/DMAs identical) so the benchmark doesn't
    ship ~33MB over the axon tunnel per call.  A tiny real output keeps the
    program well-formed."""
    key = ("nc", reps, fake_cc, tiny_io)
    if key in _CACHE:
        return _CACHE[key]
    nc = bacc.Bacc("TRN2", debug=False, num_devices=NCORES)
    heavy = "Internal" if tiny_io else "ExternalInput"
    heavy_out = "Internal" if tiny_io else "ExternalOutput"
    xpad_d = nc.dram_tensor("xpad", [NLOC, C, HP, WP], BF16, kind=heavy).ap()
    w1_d = nc.dram_tensor("w1t", [C, 9 * C], BF16, kind=heavy).ap()
    w2_d = nc.dram_tensor("w2t", [C, 9 * C], BF16, kind=heavy).ap()
    g1_d = nc.dram_tensor("gamma1", [C], F32, kind="ExternalInput").ap()
    b1_d = nc.dram_tensor("beta1", [C], F32, kind="ExternalInput").ap()
    g2_d = nc.dram_tensor("gamma2", [C], F32, kind="ExternalInput").ap()
    b2_d = nc.dram_tensor("beta2", [C], F32, kind="ExternalInput").ap()
    out_d = nc.dram_tensor("out", [NLOC, C, H, W], BF16, kind=heavy_out).ap()
    if tiny_io:
        tout_d = nc.dram_tensor("tout", [C, 1], F32, kind="ExternalOutput").ap()

    with tile.TileContext(nc) as tc:
        _build_body(tc, xpad_d, w1_d, w2_d, g1_d, b1_d, g2_d, b2_d, out_d,
                    reps=reps, fake_cc=fake_cc)
    nc.compile()
    _CACHE[key] = nc
    return nc


def _prep_in_maps(inputs):
    x = np.asarray(inputs["x"], dtype=np.float32)
    w1 = np.asarray(inputs["w1"], dtype=np.float32)
    w2 = np.asarray(inputs["w2"], dtype=np.float32)

    def wprep(w):
        wb = np.sign(w).astype(np.float32)
        return np.ascontiguousarray(
            wb.transpose(1, 2, 3, 0).reshape(C, 9 * C)).astype(NP_BF16)

    xpad = np.zeros((N, C, HP, WP), dtype=NP_BF16)
    xpad[:, :, 1:H + 1, XOFF:XOFF + W] = x.astype(NP_BF16)

    common = {
        "w1t": wprep(w1),
        "w2t": wprep(w2),
        "gamma1": np.asarray(inputs["gamma1"], np.float32),
        "beta1": np.asarray(inputs["beta1"], np.float32),
        "gamma2": np.asarray(inputs["gamma2"], np.float32),
        "beta2": np.asarray(inputs["beta2"], np.float32),
    }
    return [
        {"xpad": np.ascontiguousarray(xpad[k * NLOC:(k + 1) * NLOC]), **common}
        for k in range(NCORES)
    ]


def _run(inputs, trace=False, trace_kwargs=None, reps=1):
    in_maps = _prep_in_maps(inputs)
    nc = _build_program(reps=reps)
    res = run_bass_kernel_spmd(
        nc, in_maps, core_ids=list(range(NCORES)), trace=trace,
        **(trace_kwargs or {}))
    out = np.concatenate(
        [res.results[k]["out"].astype(np.float32) for k in range(NCORES)],
        axis=0)
    return out, res


def kernel(**inputs) -> np.ndarray:
    out, _ = _run(inputs, trace=False)
    return out
